# revision 1
# baseline (speedup 1.0000x reference)
"""AttnBlock (GroupNorm + self-attn + cross-attn + proj, residual) on 8 trn2 cores.

Sharding: data-parallel over batch B=16 -> 2 images per core; weights replicated.

v3: fp8e4 DoubleRow matmuls (K=256 per MM) for every K>=256 contraction, which
halves the PE instruction count and nearly doubles matmul throughput. All
weights are pre-scaled x16 host-side so their fp8 encoding stays out of the
subnormal range; descales ride the (otherwise free) affine slots of the PSUM
evacuation ops. GroupNorm group-reduction and broadcast run as tiny fp32
matmuls against 0/1 selector matrices (no cross-partition DMAs on the load
critical path); per-partition moments come from bn_stats. x is loaded as
bf16 (half the DMA bytes; quantization well inside tolerance). Input DMAs are
spread across the sync (x/cemb), scalar (weights), and gpsimd (bias columns)
queues so transfers overlap. Both attentions normalize after the value
matmul: the per-token 1/rowsum rides the PSUM->SBUF evacuation multiply.

Scale ledger (host WS=16 on all weights):
  qT = 2(q+bq)   kT = 2(k+bk)     -> logits' = 4*logits, exp(scale=1/64)
  v' = 2*v0 (no bias), ones_self=2 -> rinv = 1/(2r), tmp = U/r exact
  h2 = hn + tmp + bv_s (stt)
  qcT = 2(qc+bqc), kcT = 2(kc+bkc) -> exp(scale=1/64)
  ones_cross = 1/8 -> rcinv = 8/r, vc_nat = vc0+bvc, hcT = ps*rcinv = 8*hc
  proj psum = 128*(Wp hc + bp) (bias via K=1 ones matmul); y = psum/128 + x
"""

import os

import numpy as np

B, C, H, W, S, CD = 16, 256, 32, 32, 77, 512
HW = H * W
SP = 80  # S padded to a 16B-aligned stride for DoubleRow APs
GROUPS = 32
GS = C // GROUPS
EPS = 1e-5
NCORES = 8
BPC = B // NCORES

WS = 16.0          # host-side weight scale (fp8 subnormal avoidance)
QS = 2.0           # q/k/qc/kc storage scale
EXPS = 1.0 / (16.0 * QS * QS)  # exp scale: logits' = QS^2 * q.k, want q.k/16
VSC = 2.0          # v storage scale == ones_self value
HCS = 8.0          # hc storage scale; ones_cross = 1/HCS
PD = 1.0 / (WS * HCS)  # proj psum descale

_CACHE = {}
LAST_RESULT = None  # test harness reads exec_time_ns off this


def _build_nc():
    import concourse.bacc as bacc
    import concourse.bass as bass
    import concourse.tile as tile
    from concourse import mybir

    f32 = mybir.dt.float32
    bf16 = mybir.dt.bfloat16
    fp8 = mybir.dt.float8e4
    AF = mybir.ActivationFunctionType
    OP = mybir.AluOpType
    DR = mybir.MatmulPerfMode.DoubleRow

    nc = bacc.Bacc("TRN2", target_bir_lowering=False, debug=False)

    x_d = nc.dram_tensor("xbf", [BPC, C, HW], bf16, kind="ExternalInput")
    cembT_d = nc.dram_tensor("cembT", [BPC, CD // 128, 128, SP], fp8,
                             kind="ExternalInput")
    wT_d = {
        name: nc.dram_tensor(
            "wT_" + name, [kin // 128, 128, 2, 128], fp8,
            kind="ExternalInput")
        for name, kin in [("wq_s", C), ("wk_s", C), ("wv_s", C), ("wq_c", C),
                          ("w_proj", C), ("wk_c", CD), ("wv_c", CD)]
    }
    vec_d = {
        name: nc.dram_tensor(name, [C], f32, kind="ExternalInput")
        for name in [
            "gn_gamma", "gn_beta", "bq_s2", "bk_s2", "bv_s",
            "bq_c2", "bk_c2",
        ]
    }
    bp_d = nc.dram_tensor("bp_row", [1, C], bf16, kind="ExternalInput")
    bvc_d = nc.dram_tensor("bvc_row", [1, C], bf16, kind="ExternalInput")
    gsel_d = nc.dram_tensor("gsel", [128, 16], f32, kind="ExternalInput")
    gbc_d = nc.dram_tensor("gbc", [16, 128], f32, kind="ExternalInput")
    y_d = nc.dram_tensor("y", [BPC, C, HW], f32, kind="ExternalOutput")

    def bcast_ap(handle, parts):
        ap = handle[:]
        return bass.AP(tensor=ap.tensor, offset=ap.offset,
                       ap=[[0, parts]] + [list(p) for p in ap.ap])

    with tile.TileContext(nc) as tc:
        with (
            tc.tile_pool(name="const", bufs=1) as const,
            tc.tile_pool(name="work", bufs=2) as work,
            tc.tile_pool(name="psp", bufs=3, space="PSUM") as psp,
            tc.tile_pool(name="pgn", bufs=1, space="PSUM") as pgn,
            tc.tile_pool(name="pwu", bufs=1, space="PSUM") as pwu,
        ):
            # ---- constants (no DMA) ----
            ones2 = const.tile([128, 2, 128], fp8)
            nc.vector.memset(ones2, VSC)
            onesc = const.tile([S, 128], bf16)
            nc.vector.memset(onesc, 1.0 / HCS)
            ones_row = const.tile([1, 512], bf16)
            nc.vector.memset(ones_row, 1.0)
            # touch Exp once so its ACT table load overlaps the weight DMAs
            warm = const.tile([128, 1], f32)
            nc.vector.memset(warm, 0.0)
            nc.scalar.activation(warm, warm, AF.Exp)
            # dummy matmuls during the input-DMA window: keeps the PE HAM
            # activity monitor busy so real matmuls start at full clock
            dummy_mov = const.tile([128, 2, 512], fp8)
            nc.vector.memset(dummy_mov, 1.0)
            wup = pwu.tile([128, 512], f32, tag="wup", name="wup")

            def bridge(n):
                # keep-warm matmuls: no data deps, keep the PE HAM activity
                # monitor from re-throttling the clock during known PE gaps.
                # K=1/N=64 so each costs ~0.1us on the PE.
                for i in range(n):
                    nc.tensor.matmul(wup[0:32, 0:64], ones_row[0:1, 0:32],
                                     ones_row[0:1, 0:64],
                                     start=(i == 0), stop=(i == n - 1))

            def warm_burst(n):
                # sustained DR matmuls: flips the HAM to full clock (~3.4us)
                for i in range(n):
                    nc.tensor.matmul(wup, ones2, dummy_mov,
                                     start=(i == 0), stop=(i == n - 1),
                                     perf_mode=DR)

            warm_burst(9)

            # ---- input DMAs, split across queues ----
            # sync queue: x + cemb (ordered by first use)
            xTs, cembTs = [], []
            cembT0 = work.tile([128, 4, SP], fp8, tag="cembT")
            nc.sync.dma_start(out=cembT0,
                              in_=cembT_d[0].rearrange("k p s -> p k s"))
            cembTs.append(cembT0)
            xT0 = work.tile([128, 2, HW], bf16, tag="xT")
            for a in range(2):
                nc.sync.dma_start(
                    out=xT0[:, a, :],
                    in_=x_d[0].rearrange("(a p) n -> p a n", p=128)[:, a, :])
            xTs.append(xT0)
            cembT1 = work.tile([128, 4, SP], fp8, tag="cembT")
            nc.sync.dma_start(out=cembT1,
                              in_=cembT_d[1].rearrange("k p s -> p k s"))
            cembTs.append(cembT1)
            xT1 = work.tile([128, 2, HW], bf16, tag="xT")
            for a in range(2):
                nc.sync.dma_start(
                    out=xT1[:, a, :],
                    in_=x_d[1].rearrange("(a p) n -> p a n", p=128)[:, a, :])
            xTs.append(xT1)

            # scalar queue: weights + selectors + proj bias row
            wT = {}

            def load_w(name, kin):
                kch = kin // 128
                wt = const.tile([128, kch, 2, 128], fp8, tag=f"wT_{name}")
                nc.scalar.dma_start(
                    out=wt, in_=wT_d[name][:].rearrange("k p m c -> p k m c"))
                wT[name] = wt

            load_w("wk_c", CD)
            load_w("wv_c", CD)
            gsel = const.tile([128, 16], f32)
            nc.scalar.dma_start(out=gsel, in_=gsel_d[:])
            gbc = const.tile([16, 128], f32)
            nc.scalar.dma_start(out=gbc, in_=gbc_d[:])
            load_w("wq_s", C)
            load_w("wk_s", C)
            load_w("wv_s", C)
            load_w("wq_c", C)
            load_w("w_proj", C)
            bvc_sb = const.tile([1, C], bf16)
            nc.scalar.dma_start(out=bvc_sb, in_=bvc_d[:])
            bp_sb = const.tile([1, C], bf16)
            nc.scalar.dma_start(out=bp_sb, in_=bp_d[:])

            # gpsimd queue: bias/affine columns (ordered by first use)
            cols = {}
            for name in ["bk_c2", "gn_gamma", "gn_beta", "bq_s2",
                         "bk_s2", "bq_c2", "bv_s"]:
                t = const.tile([128, 2], f32, tag=f"col_{name}")
                nc.gpsimd.dma_start(
                    out=t, in_=vec_d[name][:].rearrange("(a p) -> p a", p=128))
                cols[name] = t

            wvs_flat = wT["wv_s"][:].rearrange("p k m c -> p k (m c)")
            wvc_flat = wT["wv_c"][:].rearrange("p k m c -> p k (m c)")

            nb = lambda ap, nh: ap[:, nh * 512:(nh + 1) * 512]

            # per-image tiles
            T = [dict(xT=xTs[b], cembT=cembTs[b]) for b in range(BPC)]
            for b in range(BPC):
                t = T[b]
                for key, shape, dt_ in [
                    ("kcT", [128, 2, SP], fp8),
                    ("vc_nat", [S, C], bf16),
                    ("stats6", [128, 2, 2, 6], f32),
                    ("qsum", [128, 2, 2], f32),
                    ("m2sum", [128, 2, 2], f32),
                    ("msq_e", [128, 2, 2], f32),
                    ("musq", [128, 2, 2], f32),
                    ("spack", [128, 3, 2, 1], f32),
                    ("tm", [16, 2], f32),
                    ("ex2", [16, 2], f32),
                    ("msq", [16, 2], f32),
                    ("varv", [16, 2], f32),
                    ("ya", [16, 2], f32),
                    ("yb", [16, 2], f32),
                    ("y2", [16, 2], f32),
                    ("mrp", [16, 4], f32),
                    ("Acol", [128, 2], f32),
                    ("Bcol", [128, 2], f32),
                    ("t1", [128, 2], f32),
                    ("hnmm", [128, 2, HW], fp8),
                    ("qT", [128, 2, HW], fp8),
                    ("kT", [128, 2, HW], fp8),
                    ("v_nat", [128, 8, C], fp8),
                    ("expST", [128, 8, HW], fp8),
                    ("rinv", [128, HW], f32),
                    ("tmp", [128, 2, HW], bf16),
                    ("h2T", [128, 2, HW], fp8),
                    ("qcT", [128, 2, HW], fp8),
                    ("expScT", [S, HW], bf16),
                    ("rcinv", [128, HW], f32),
                    ("hcT", [128, 2, HW], fp8),
                    ("y_sb", [128, 2, HW], f32),
                ]:
                    t[key] = work.tile(shape, dt_, tag=key, name=key)

            ps = lambda: psp.tile([128, HW], f32, tag="ps", name="ps")
            def gn_stats(b):
                # bn_stats (DVE) + per-partition packed moments
                t = T[b]
                AX = mybir.AxisListType
                for a in range(2):
                    for ch in range(2):
                        nc.vector.bn_stats(
                            t["stats6"][:, a, ch, :],
                            t["xT"][:, a, ch * 512:(ch + 1) * 512])
                s6 = t["stats6"]
                m_e, m_o = s6[:, :, :, 1:2], s6[:, :, :, 4:5]
                v_e, v_o = s6[:, :, :, 2:3], s6[:, :, :, 5:6]
                nc.vector.tensor_add(t["qsum"], m_e, m_o)
                nc.vector.tensor_add(t["m2sum"], v_e, v_o)
                nc.vector.tensor_mul(t["msq_e"], m_e, m_e)
                nc.vector.tensor_mul(t["musq"], m_o, m_o)
                nc.vector.tensor_add(t["musq"], t["musq"], t["msq_e"])
                nc.vector.reduce_sum(out=t["spack"][:, 0, :, :],
                                     in_=t["qsum"], axis=AX.X)
                nc.vector.reduce_sum(out=t["spack"][:, 1, :, :],
                                     in_=t["m2sum"], axis=AX.X)
                nc.vector.reduce_sum(out=t["spack"][:, 2, :, :],
                                     in_=t["musq"], axis=AX.X)
            def gn_group(b):
                # group reduce via PE selector matmul; mean/var/rstd on [16,2]
                t = T[b]
                gps = pgn.tile([128, 512], f32, tag="gps", name="gps")
                nc.tensor.matmul(gps[0:16, 0:6], gsel, t["spack"],
                                 start=True, stop=True)
                nc.vector.tensor_scalar_mul(t["mrp"][:, 0:2], gps[0:16, 0:2],
                                            1.0 / 32.0)
                nc.vector.tensor_scalar_mul(t["tm"], gps[0:16, 2:4],
                                            1.0 / 8192.0)
                nc.vector.scalar_tensor_tensor(
                    out=t["ex2"], in0=gps[0:16, 4:6], scalar=1.0 / 32.0,
                    in1=t["tm"], op0=OP.mult, op1=OP.add)
                nc.vector.tensor_mul(t["msq"], t["mrp"][:, 0:2],
                                     t["mrp"][:, 0:2])
                nc.vector.tensor_sub(t["varv"], t["ex2"], t["msq"])
                nc.vector.tensor_scalar_add(t["varv"], t["varv"], EPS)
                nc.vector.reciprocal_approx_fast(out=t["ya"], in_=t["varv"])
                cur = t["ya"]
                for it in range(1):
                    nc.vector.tensor_mul(t["y2"], cur, cur)
                    nc.vector.tensor_mul(t["y2"], t["y2"], t["varv"])
                    nc.vector.tensor_scalar(out=t["y2"], in0=t["y2"],
                                            scalar1=-0.5, scalar2=1.5,
                                            op0=OP.mult, op1=OP.add)
                    nxt = t["yb"] if cur is t["ya"] else t["ya"]
                    nc.vector.tensor_mul(nxt, cur, t["y2"])
                    cur = nxt
                nc.vector.tensor_copy(t["mrp"][:, 2:4], cur)

            def gn_bcast(b):
                t = T[b]
                mps = pgn.tile([128, 512], f32, tag="gps", name="mps")
                nc.tensor.matmul(mps[0:128, 0:4], gbc, t["mrp"],
                                 start=True, stop=True)
                t["mps"] = mps

            def gn_affine(b):
                t = T[b]
                mps = t["mps"]
                nc.vector.tensor_mul(t["Acol"], mps[0:128, 2:4],
                                     cols["gn_gamma"])
                nc.vector.tensor_mul(t["t1"], mps[0:128, 0:2], t["Acol"])
                nc.vector.tensor_sub(t["Bcol"], cols["gn_beta"], t["t1"])
                for a in range(2):
                    nc.vector.tensor_scalar(
                        out=t["hnmm"][:, a, :], in0=t["xT"][:, a, :],
                        scalar1=t["Acol"][:, a:a + 1],
                        scalar2=t["Bcol"][:, a:a + 1],
                        op0=OP.mult, op1=OP.add)

            def qkv(b):
                # q/k, v projections for one image. Image 0's PSUM
                # evacuations ride ACT (idle then); image 1's ride DVE so
                # they don't queue behind image 0's exps on the ACT FIFO.
                t = T[b]
                for wname, bname, dst in [("wq_s", "bq_s2", t["qT"]),
                                          ("wk_s", "bk_s2", t["kT"])]:
                    for mc in range(2):
                        qp = ps()
                        for nh in range(2):
                            nc.tensor.matmul(
                                nb(qp, nh), wT[wname][:, :, mc, :],
                                t["hnmm"][:, :, nh * 512:(nh + 1) * 512],
                                start=True, stop=True, perf_mode=DR)
                        if b == 0:
                            nc.scalar.activation(
                                out=dst[:, mc, :], in_=qp, func=AF.Identity,
                                bias=cols[bname][:, mc:mc + 1], scale=QS / WS)
                        else:
                            nc.vector.tensor_scalar(
                                out=dst[:, mc, :], in0=qp, scalar1=QS / WS,
                                scalar2=cols[bname][:, mc:mc + 1],
                                op0=OP.mult, op1=OP.add)
                for half in range(2):
                    vp = ps()
                    for j in range(4):
                        m8 = 4 * half + j
                        nc.tensor.matmul(
                            vp[:, j * 256:(j + 1) * 256],
                            t["hnmm"][:, :, m8 * 128:(m8 + 1) * 128],
                            wvs_flat,
                            start=True, stop=True, perf_mode=DR)
                    vdst = t["v_nat"][:, 4 * half:4 * half + 4, :]
                    vsrc = vp[:].rearrange("p (j c) -> p j c", c=256)
                    if b == 0:
                        nc.scalar.mul(vdst, vsrc, VSC / WS)
                    else:
                        nc.vector.tensor_scalar_mul(vdst, vsrc, VSC / WS)

            def spexp_one(b, m8):
                t = T[b]
                sp = ps()
                for nh in range(2):
                    nc.tensor.matmul(
                        nb(sp, nh), t["kT"][:, :, m8 * 128:(m8 + 1) * 128],
                        t["qT"][:, :, nh * 512:(nh + 1) * 512],
                        start=True, stop=True, perf_mode=DR)
                nc.scalar.activation(t["expST"][:, m8, :], sp, AF.Exp,
                                     scale=EXPS)

            def spexp_both():
                # interleave the two images' S^T+exp streams (image 0 ahead)
                # so the ACT exp pipeline never drains between images
                order = [(b, m8) for b in range(BPC) for m8 in range(8)]
                for b, m8 in order:
                    spexp_one(b, m8)

            def stage_rsum(b):
                t = T[b]
                rp = ps()
                for nh in range(2):
                    for i in range(4):
                        nc.tensor.matmul(
                            nb(rp, nh), ones2,
                            t["expST"][:, 2 * i:2 * i + 2,
                                       nh * 512:(nh + 1) * 512],
                            start=(i == 0), stop=(i == 3), perf_mode=DR)
                nc.vector.reciprocal_approx_fast(out=t["rinv"], in_=rp)

            def av_half(b, mc):
                # attnV for one output-channel half: single PSUM tile so it
                # can interleave with the exp-paced S^T stream of the other
                # image without exhausting the pool
                t = T[b]
                ap2 = ps()
                for i in range(4):
                    for nh in range(2):
                        nc.tensor.matmul(
                            nb(ap2, nh),
                            t["v_nat"][:, 2 * i:2 * i + 2,
                                       mc * 128:(mc + 1) * 128],
                            t["expST"][:, 2 * i:2 * i + 2,
                                       nh * 512:(nh + 1) * 512],
                            start=(i == 0), stop=(i == 3), perf_mode=DR)
                nc.vector.tensor_tensor(t["tmp"][:, mc, :], ap2,
                                        t["rinv"], op=OP.mult)
                nc.vector.scalar_tensor_tensor(
                    out=t["h2T"][:, mc, :], in0=t["tmp"][:, mc, :],
                    scalar=cols["bv_s"][:, mc:mc + 1],
                    in1=t["hnmm"][:, mc, :], op0=OP.add, op1=OP.add)

            def c_qc(b, nh):
                t = T[b]
                qp = ps()
                for mc in range(2):
                    nc.tensor.matmul(
                        qp[:, mc * 512:(mc + 1) * 512],
                        wT["wq_c"][:, :, mc, :],
                        t["h2T"][:, :, nh * 512:(nh + 1) * 512],
                        start=True, stop=True, perf_mode=DR)
                for mc in range(2):
                    nc.scalar.activation(
                        out=t["qcT"][:, mc, nh * 512:(nh + 1) * 512],
                        in_=qp[:, mc * 512:(mc + 1) * 512], func=AF.Identity,
                        bias=cols["bq_c2"][:, mc:mc + 1], scale=QS / WS)

            def c_sc(b, nh):
                t = T[b]
                scp = ps()
                nc.tensor.matmul(
                    scp[0:SP, 0:512], t["kcT"][:],
                    t["qcT"][:, :, nh * 512:(nh + 1) * 512],
                    start=True, stop=True, perf_mode=DR)
                nc.scalar.activation(
                    t["expScT"][:, nh * 512:(nh + 1) * 512],
                    scp[0:S, 0:512], AF.Exp, scale=EXPS)

            def c_crhc(b, nh):
                t = T[b]
                esl = t["expScT"][:, nh * 512:(nh + 1) * 512]
                crp = ps()
                nc.tensor.matmul(crp[:, 0:512], onesc, esl,
                                 start=True, stop=True)
                rsl = t["rcinv"][:, nh * 512:(nh + 1) * 512]
                nc.vector.reciprocal_approx_fast(out=rsl, in_=crp[:, 0:512])
                hcp = ps()
                for mc in range(2):
                    nc.tensor.matmul(
                        hcp[:, mc * 512:(mc + 1) * 512],
                        t["vc_nat"][:, mc * 128:(mc + 1) * 128], esl,
                        start=True, stop=True)
                rbc = bass.AP(tensor=rsl.tensor, offset=rsl.offset,
                              ap=[list(rsl.ap[0]), [0, 2],
                                  list(rsl.ap[1])])
                nc.vector.tensor_tensor(
                    t["hcT"][:, :, nh * 512:(nh + 1) * 512],
                    hcp[:].rearrange("p (m n) -> p m n", n=512),
                    rbc, op=OP.mult)

            def c_proj(b, nh):
                t = T[b]
                pp = ps()
                for mc in range(2):
                    nc.tensor.matmul(
                        pp[:, mc * 512:(mc + 1) * 512],
                        bp_sb[0:1, mc * 128:(mc + 1) * 128], ones_row[0:1, :],
                        start=True, stop=False, skip_group_check=True)
                    nc.tensor.matmul(
                        pp[:, mc * 512:(mc + 1) * 512],
                        wT["w_proj"][:, :, mc, :],
                        t["hcT"][:, :, nh * 512:(nh + 1) * 512],
                        start=False, stop=True, perf_mode=DR,
                        skip_group_check=True)
                nc.vector.scalar_tensor_tensor(
                    out=t["y_sb"][:, :, nh * 512:(nh + 1) * 512],
                    in0=pp[:].rearrange("p (m n) -> p m n", n=512),
                    scalar=PD,
                    in1=t["xT"][:, :, nh * 512:(nh + 1) * 512],
                    op0=OP.mult, op1=OP.add)
                eng = nc.sync if nh == 0 else nc.scalar
                eng.dma_start(
                    out=y_d[b].rearrange("(a p) n -> p a n",
                                         p=128)[:, :, nh * 512:(nh + 1) * 512],
                    in_=t["y_sb"][:, :, nh * 512:(nh + 1) * 512])


            gn_stats(0)
            # ================= stage B: cross k/v matmuls ==============
            kc_pss, vc_pss = [], []
            for b in range(BPC):
                t = T[b]
                kc_ps = ps()
                kc_pss.append(kc_ps)
                for mc in range(2):
                    for i in range(2):
                        nc.tensor.matmul(
                            kc_ps[:, mc * 512:mc * 512 + SP],
                            wT["wk_c"][:, 2 * i:2 * i + 2, mc, :],
                            t["cembT"][:, 2 * i:2 * i + 2, :],
                            start=(i == 0), stop=(i == 1), perf_mode=DR)
                vc_ps = ps()
                vc_pss.append(vc_ps)
                nc.tensor.matmul(
                    vc_ps[0:SP, 0:C], ones_row[0:1, 0:SP], bvc_sb[0:1, :],
                    start=True, stop=False, skip_group_check=True)
                for i in range(2):
                    nc.tensor.matmul(
                        vc_ps[0:SP, 0:C],
                        t["cembT"][:, 2 * i:2 * i + 2, :],
                        wvc_flat[:, 2 * i:2 * i + 2, :],
                        start=False, stop=(i == 1), perf_mode=DR,
                        skip_group_check=True)
            for b in range(BPC):
                t = T[b]
                nc.vector.memset(t["kcT"][:, :, S:SP], 0.0)
                for mc in range(2):
                    nc.scalar.activation(
                        out=t["kcT"][:, mc, 0:S],
                        in_=kc_pss[b][:, mc * 512:mc * 512 + S],
                        func=AF.Identity,
                        bias=cols["bk_c2"][:, mc:mc + 1], scale=QS / WS)
                nc.scalar.mul(t["vc_nat"], vc_pss[b][0:S, 0:C], 1.0 / WS)

            gn_stats(1)
            gn_group(0)
            warm_burst(12)
            gn_group(1)
            gn_bcast(0)
            gn_affine(0)
            warm_burst(3)
            gn_bcast(1)
            gn_affine(1)
            qkv(0)
            qkv(1)
            warm_burst(3)
            for m8 in range(8):
                spexp_one(0, m8)
            for m8 in range(3):
                spexp_one(1, m8)
            stage_rsum(0)
            spexp_one(1, 3)
            av_half(0, 0)
            spexp_one(1, 4)
            spexp_one(1, 5)
            av_half(0, 1)
            spexp_one(1, 6)
            spexp_one(1, 7)
            c_qc(0, 0)
            c_qc(0, 1)
            stage_rsum(1)
            av_half(1, 0)
            av_half(1, 1)
            c_qc(1, 0)
            c_sc(0, 0)
            c_qc(1, 1)
            c_sc(0, 1)
            c_crhc(0, 0)
            c_sc(1, 0)
            c_crhc(0, 1)
            c_proj(0, 0)
            c_sc(1, 1)
            c_crhc(1, 0)
            c_proj(0, 1)
            c_crhc(1, 1)
            c_proj(1, 0)
            warm_burst(3)
            c_proj(1, 1)

    nc.finalize()
    return nc


def host_inputs(inputs):
    import ml_dtypes
    bf16 = ml_dtypes.bfloat16
    fp8 = ml_dtypes.float8_e4m3
    f = lambda a: np.ascontiguousarray(np.asarray(a, dtype=np.float32))
    x = f(inputs["x"]).reshape(B, C, HW).astype(bf16)
    cembT = np.zeros((B, CD // 128, 128, SP), np.float32)
    cembT[:, :, :, :S] = f(inputs["cemb"]).transpose(0, 2, 1).reshape(
        B, CD // 128, 128, S)
    cembT = cembT.astype(fp8)
    gsel = np.zeros((128, 16), np.float32)
    gsel[np.arange(128), np.arange(128) // 8] = 1.0
    shared = {
        "gn_gamma": f(inputs["gn_gamma"]),
        "gn_beta": f(inputs["gn_beta"]),
        "bv_s": f(inputs["bv_s"]),
        "bvc_row": np.ascontiguousarray(
            (WS * f(inputs["bv_c"])).reshape(1, C)).astype(bf16),
        "bq_s2": QS * f(inputs["bq_s"]),
        "bk_s2": QS * f(inputs["bk_s"]),
        "bq_c2": QS * f(inputs["bq_c"]),
        "bk_c2": QS * f(inputs["bk_c"]),
        "bp_row": np.ascontiguousarray(
            (WS * HCS * f(inputs["b_proj"])).reshape(1, C)).astype(bf16),
        "gsel": gsel,
        "gbc": np.ascontiguousarray(gsel.T),
    }
    for name in ["wq_s", "wk_s", "wv_s", "wq_c", "w_proj", "wk_c", "wv_c"]:
        w = f(inputs[name])
        kin = w.shape[1]
        shared["wT_" + name] = np.ascontiguousarray(
            (WS * w.T).reshape(kin // 128, 128, 2, 128)).astype(fp8)
    return [
        {"xbf": x[i * BPC:(i + 1) * BPC],
         "cembT": cembT[i * BPC:(i + 1) * BPC], **shared}
        for i in range(NCORES)
    ]


def kernel(**inputs):
    global LAST_RESULT
    from concourse.bass_utils import run_bass_kernel_spmd

    if "nc" not in _CACHE:
        _CACHE["nc"] = _build_nc()
    nc = _CACHE["nc"]

    in_maps = host_inputs(inputs)
    res = run_bass_kernel_spmd(nc, in_maps, list(range(NCORES)),
                               trace=bool(os.environ.get("BASS_TRACE")))
    LAST_RESULT = res
    y = np.concatenate([res.results[i]["y"] for i in range(NCORES)], axis=0)
    return y.reshape(B, C, H, W).astype(np.float32)



# revision 3
# speedup vs baseline: 1.1014x; 1.1014x over previous
"""AttnBlock (GroupNorm + self-attn + cross-attn + proj, residual) on 8 trn2 cores.

Sharding: data-parallel over batch B=16 -> 2 images per core; weights replicated.

v4: algebraic restructuring on top of v3's fp8 DoubleRow pipeline.
 - proj folded into the value tensor: softmax rows sum to 1, so
   out = (E/r) @ (vc @ Wp^T + bp). The [1024,256] proj becomes a [77,256]
   "vcp" computed right after vc, off the critical path; the cross attnV
   matmul then produces the final output directly.
 - h2 never materialized: it only feeds qc, so qc = Wqc hn + Wqc (U/r)
   via two-moving-tensor PSUM accumulation; bias bqc + Wqc bv_s is host-
   precomputed. Removes the ADD,ADD residual chain from DVE.
 - q/k merged: S = (hn M2 + bq Wk) hn^T with M2 = Wq^T Wk host-computed
   (bias terms constant over keys are softmax-invariant and dropped).
   One g-matmul replaces the q and k matmuls; stationary for S^T is hnmm.
 - cross softmax normalization moves onto the small [77,512] exp tile
   (escn = E * rcinv) so the attnV psum is final up to one scalar.
 - y stored bf16 (half the drain DMA bytes); host converts to f32.

Scale ledger (host WS=16 on true weights):
  M2' = GMS*(Wq^T Wk), GMS=128; gT = GQS*g + GQS*(bq Wk), GQS=8
  S^T psum = GQS*logits -> exp(scale=1/(16*GQS))
  v' = VSC*(hn Wv^T), VSC=2 = ones_self -> rinv = 1/(VSC*r), tmp = U/r fp8
  qc psum = WS*(Wqc(hn+tmp)); qcT = QS*qc + QS*(bqc + Wqc bv_s), QS=2
  kcT = QS*kc; sc psum = QS^2*(qc.kc) -> exp(scale=1/64), expScT bf16
  vcT = VS2*vc (VS2=4, bias col VS2*bvc); vcp psum = VS2*WS*(vc Wp^T)+bias
  vcp bf16 natural (evac scale 1/(VS2*WS)); onesc = 1/HCS (HCS=8)
  rcinv = HCS/rc; escn = E*rcinv; hc psum = HCS*out; y = psum/HCS + x
"""

import os

import numpy as np

B, C, H, W, S, CD = 16, 256, 32, 32, 77, 512
HW = H * W
SP = 80  # S padded to a 16B-aligned stride for DoubleRow APs
GROUPS = 32
GS = C // GROUPS
EPS = 1e-5
NCORES = 8
BPC = B // NCORES

WS = 16.0          # host-side weight scale (fp8 subnormal avoidance)
QS = 2.0           # qc/kc storage scale
GMS = 128.0        # host scale on M2 = Wq^T Wk
GQS = 8.0          # gT storage scale
EXPS_S = 1.0 / (16.0 * GQS)    # self exp scale
EXPS_C = 1.0 / (16.0 * QS * QS)  # cross exp scale
VSC = 2.0          # v storage scale == ones_self value
VS2 = 4.0          # vc fp8 storage scale
HCS = 8.0          # ones_cross = 1/HCS; final evac scale 1/HCS

_CACHE = {}
LAST_RESULT = None  # test harness reads exec_time_ns off this


def _build_nc():
    import concourse.bacc as bacc
    import concourse.bass as bass
    import concourse.tile as tile
    from concourse import mybir

    f32 = mybir.dt.float32
    bf16 = mybir.dt.bfloat16
    fp8 = mybir.dt.float8e4
    AF = mybir.ActivationFunctionType
    OP = mybir.AluOpType
    DR = mybir.MatmulPerfMode.DoubleRow

    nc = bacc.Bacc("TRN2", target_bir_lowering=False, debug=False)

    x_d = nc.dram_tensor("xbf", [BPC, C, HW], bf16, kind="ExternalInput")
    cembT_d = nc.dram_tensor("cembT", [BPC, CD // 128, 128, SP], fp8,
                             kind="ExternalInput")
    wT_d = {
        name: nc.dram_tensor(
            "wT_" + name, [kin // 128, 128, 2, 128], fp8,
            kind="ExternalInput")
        for name, kin in [("m2", C), ("wv_s", C), ("wq_c", C),
                          ("w_proj", C), ("wk_c", CD), ("wv_c", CD)]
    }
    vec_d = {
        name: nc.dram_tensor(name, [C], f32, kind="ExternalInput")
        for name in [
            "gn_gamma", "gn_beta", "bg2", "bq_c2", "bk_c2", "bvc_col",
        ]
    }
    bp_d = nc.dram_tensor("bp_row", [1, C], bf16, kind="ExternalInput")
    gsel_d = nc.dram_tensor("gsel", [128, 16], f32, kind="ExternalInput")
    gbc_d = nc.dram_tensor("gbc", [16, 128], f32, kind="ExternalInput")
    y_d = nc.dram_tensor("y", [BPC, C, HW], bf16, kind="ExternalOutput")

    with tile.TileContext(nc) as tc:
        with (
            tc.tile_pool(name="const", bufs=1) as const,
            tc.tile_pool(name="work", bufs=2) as work,
            tc.tile_pool(name="psp", bufs=3, space="PSUM") as psp,
            tc.tile_pool(name="pgn", bufs=1, space="PSUM") as pgn,
            tc.tile_pool(name="pwu", bufs=1, space="PSUM") as pwu,
        ):
            # ---- constants (no DMA) ----
            ones2 = const.tile([128, 2, 128], fp8)
            nc.vector.memset(ones2, VSC)
            onesc = const.tile([S, 128], bf16)
            nc.vector.memset(onesc, 1.0 / HCS)
            ones_row = const.tile([1, 512], bf16)
            nc.vector.memset(ones_row, 1.0)
            # touch Exp once so its ACT table load overlaps the weight DMAs
            warm = const.tile([128, 1], f32)
            nc.vector.memset(warm, 0.0)
            nc.scalar.activation(warm, warm, AF.Exp)
            # dummy matmuls during the input-DMA window: keeps the PE HAM
            # activity monitor busy so real matmuls start at full clock
            dummy_mov = const.tile([128, 2, 512], fp8)
            nc.vector.memset(dummy_mov, 1.0)
            wup = pwu.tile([128, 512], f32, tag="wup", name="wup")

            def warm_burst(n):
                # sustained DR matmuls: flips the HAM to full clock
                for i in range(n):
                    nc.tensor.matmul(wup, ones2, dummy_mov,
                                     start=(i == 0), stop=(i == n - 1),
                                     perf_mode=DR)

            warm_burst(9)

            # ---- input DMAs, split across queues ----
            # sync queue: cemb (tiny, needed first by stage B) then x chunks
            xTs, cembTs = [], []
            for b in range(BPC):
                cembT = work.tile([128, 4, SP], fp8, tag="cembT")
                nc.sync.dma_start(out=cembT,
                                  in_=cembT_d[b].rearrange("k p s -> p k s"))
                cembTs.append(cembT)
            for b in range(BPC):
                xT = work.tile([128, 2, HW], bf16, tag="xT")
                for a in range(2):
                    for ch in range(2):
                        nc.sync.dma_start(
                            out=xT[:, a, ch * 512:(ch + 1) * 512],
                            in_=x_d[b].rearrange(
                                "(a p) n -> p a n",
                                p=128)[:, a, ch * 512:(ch + 1) * 512])
                xTs.append(xT)

            # scalar queue: weights + selectors (ordered by first use)
            wT = {}

            def load_w(name, kin):
                kch = kin // 128
                wt = const.tile([128, kch, 2, 128], fp8, tag=f"wT_{name}")
                nc.scalar.dma_start(
                    out=wt, in_=wT_d[name][:].rearrange("k p m c -> p k m c"))
                wT[name] = wt

            load_w("wk_c", CD)
            load_w("wv_c", CD)
            gsel = const.tile([128, 16], f32)
            nc.scalar.dma_start(out=gsel, in_=gsel_d[:])
            gbc = const.tile([16, 128], f32)
            nc.scalar.dma_start(out=gbc, in_=gbc_d[:])
            load_w("w_proj", C)
            bp_sb = const.tile([1, C], bf16)
            nc.scalar.dma_start(out=bp_sb, in_=bp_d[:])
            load_w("m2", C)
            load_w("wv_s", C)
            load_w("wq_c", C)

            # gpsimd queue: bias/affine columns (ordered by first use)
            cols = {}
            for name in ["bk_c2", "bvc_col", "gn_gamma", "gn_beta",
                         "bg2", "bq_c2"]:
                t = const.tile([128, 2], f32, tag=f"col_{name}")
                nc.gpsimd.dma_start(
                    out=t, in_=vec_d[name][:].rearrange("(a p) -> p a", p=128))
                cols[name] = t

            wvs_flat = wT["wv_s"][:].rearrange("p k m c -> p k (m c)")
            wproj_flat = wT["w_proj"][:].rearrange("p k m c -> p k (m c)")

            nb = lambda ap, nh: ap[:, nh * 512:(nh + 1) * 512]

            # per-image tiles
            T = [dict(xT=xTs[b], cembT=cembTs[b]) for b in range(BPC)]
            for b in range(BPC):
                t = T[b]
                for key, shape, dt_ in [
                    ("kcT", [128, 2, SP], fp8),
                    ("vc_f8", [128, 2, SP], fp8),
                    ("vcp", [S, C], bf16),
                    ("stats6", [128, 2, 2, 6], f32),
                    ("qsum", [128, 2, 2], f32),
                    ("m2sum", [128, 2, 2], f32),
                    ("msq_e", [128, 2, 2], f32),
                    ("musq", [128, 2, 2], f32),
                    ("spack", [128, 3, 2, 1], f32),
                    ("tm", [16, 2], f32),
                    ("ex2", [16, 2], f32),
                    ("msq", [16, 2], f32),
                    ("varv", [16, 2], f32),
                    ("ya", [16, 2], f32),
                    ("yb", [16, 2], f32),
                    ("y2", [16, 2], f32),
                    ("mrp", [16, 4], f32),
                    ("Acol", [128, 2], f32),
                    ("Bcol", [128, 2], f32),
                    ("t1", [128, 2], f32),
                    ("hnmm", [128, 2, HW], fp8),
                    ("gT", [128, 2, HW], fp8),
                    ("v_nat", [128, 8, C], fp8),
                    ("expST", [128, 8, HW], fp8),
                    ("rinv", [128, HW], f32),
                    ("tmp", [128, 2, HW], fp8),
                    ("qcT", [128, 2, HW], fp8),
                    ("expScT", [S, HW], bf16),
                    ("escn", [S, HW], bf16),
                    ("rcinv", [128, HW], f32),
                    ("y_sb", [128, 2, HW], bf16),
                ]:
                    t[key] = work.tile(shape, dt_, tag=key, name=key)

            ps = lambda: psp.tile([128, HW], f32, tag="ps", name="ps")

            def gn_stats(b):
                # bn_stats (DVE) + per-partition packed moments
                t = T[b]
                AX = mybir.AxisListType
                for a in range(2):
                    for ch in range(2):
                        nc.vector.bn_stats(
                            t["stats6"][:, a, ch, :],
                            t["xT"][:, a, ch * 512:(ch + 1) * 512])
                s6 = t["stats6"]
                m_e, m_o = s6[:, :, :, 1:2], s6[:, :, :, 4:5]
                v_e, v_o = s6[:, :, :, 2:3], s6[:, :, :, 5:6]
                nc.vector.tensor_add(t["qsum"], m_e, m_o)
                nc.vector.tensor_add(t["m2sum"], v_e, v_o)
                nc.vector.tensor_mul(t["msq_e"], m_e, m_e)
                nc.vector.tensor_mul(t["musq"], m_o, m_o)
                nc.vector.tensor_add(t["musq"], t["musq"], t["msq_e"])
                nc.vector.reduce_sum(out=t["spack"][:, 0, :, :],
                                     in_=t["qsum"], axis=AX.X)
                nc.vector.reduce_sum(out=t["spack"][:, 1, :, :],
                                     in_=t["m2sum"], axis=AX.X)
                nc.vector.reduce_sum(out=t["spack"][:, 2, :, :],
                                     in_=t["musq"], axis=AX.X)

            def gn_group(b):
                # group reduce via PE selector matmul; mean/var/rstd on [16,2]
                t = T[b]
                gps = pgn.tile([128, 512], f32, tag="gps", name="gps")
                nc.tensor.matmul(gps[0:16, 0:6], gsel, t["spack"],
                                 start=True, stop=True)
                nc.vector.tensor_scalar_mul(t["mrp"][:, 0:2], gps[0:16, 0:2],
                                            1.0 / 32.0)
                nc.vector.tensor_scalar_mul(t["tm"], gps[0:16, 2:4],
                                            1.0 / 8192.0)
                nc.vector.scalar_tensor_tensor(
                    out=t["ex2"], in0=gps[0:16, 4:6], scalar=1.0 / 32.0,
                    in1=t["tm"], op0=OP.mult, op1=OP.add)
                nc.vector.tensor_mul(t["msq"], t["mrp"][:, 0:2],
                                     t["mrp"][:, 0:2])
                nc.vector.tensor_sub(t["varv"], t["ex2"], t["msq"])
                nc.vector.tensor_scalar_add(t["varv"], t["varv"], EPS)
                nc.vector.reciprocal_approx_fast(out=t["ya"], in_=t["varv"])
                cur = t["ya"]
                for it in range(1):
                    nc.vector.tensor_mul(t["y2"], cur, cur)
                    nc.vector.tensor_mul(t["y2"], t["y2"], t["varv"])
                    nc.vector.tensor_scalar(out=t["y2"], in0=t["y2"],
                                            scalar1=-0.5, scalar2=1.5,
                                            op0=OP.mult, op1=OP.add)
                    nxt = t["yb"] if cur is t["ya"] else t["ya"]
                    nc.vector.tensor_mul(nxt, cur, t["y2"])
                    cur = nxt
                nc.vector.tensor_copy(t["mrp"][:, 2:4], cur)

            def gn_bcast(b):
                t = T[b]
                mps = pgn.tile([128, 512], f32, tag="gps", name="mps")
                nc.tensor.matmul(mps[0:128, 0:4], gbc, t["mrp"],
                                 start=True, stop=True)
                t["mps"] = mps

            def gn_affine(b):
                t = T[b]
                mps = t["mps"]
                nc.vector.tensor_mul(t["Acol"], mps[0:128, 2:4],
                                     cols["gn_gamma"])
                nc.vector.tensor_mul(t["t1"], mps[0:128, 0:2], t["Acol"])
                nc.vector.tensor_sub(t["Bcol"], cols["gn_beta"], t["t1"])
                for a in range(2):
                    if b == 0:
                        nc.scalar.activation(
                            out=t["hnmm"][:, a, :], in_=t["xT"][:, a, :],
                            func=AF.Identity,
                            bias=t["Bcol"][:, a:a + 1],
                            scale=t["Acol"][:, a:a + 1])
                    else:
                        nc.vector.tensor_scalar(
                            out=t["hnmm"][:, a, :], in0=t["xT"][:, a, :],
                            scalar1=t["Acol"][:, a:a + 1],
                            scalar2=t["Bcol"][:, a:a + 1],
                            op0=OP.mult, op1=OP.add)

            def gv(b):
                # g = hn M2 + bg (one matmul replaces q and k), then v.
                # Image 0's PSUM evacuations ride ACT (idle then); image 1's
                # ride DVE so they don't queue behind image 0's exps.
                t = T[b]
                for mc in range(2):
                    qp = ps()
                    for nh in range(2):
                        nc.tensor.matmul(
                            nb(qp, nh), wT["m2"][:, :, mc, :],
                            t["hnmm"][:, :, nh * 512:(nh + 1) * 512],
                            start=True, stop=True, perf_mode=DR)
                    if b == 0:
                        nc.scalar.activation(
                            out=t["gT"][:, mc, :], in_=qp, func=AF.Identity,
                            bias=cols["bg2"][:, mc:mc + 1], scale=GQS / GMS)
                    else:
                        nc.vector.tensor_scalar(
                            out=t["gT"][:, mc, :], in0=qp, scalar1=GQS / GMS,
                            scalar2=cols["bg2"][:, mc:mc + 1],
                            op0=OP.mult, op1=OP.add)
                for half in range(2):
                    vp = ps()
                    for j in range(4):
                        m8 = 4 * half + j
                        nc.tensor.matmul(
                            vp[:, j * 256:(j + 1) * 256],
                            t["hnmm"][:, :, m8 * 128:(m8 + 1) * 128],
                            wvs_flat,
                            start=True, stop=True, perf_mode=DR)
                    vdst = t["v_nat"][:, 4 * half:4 * half + 4, :]
                    vsrc = vp[:].rearrange("p (j c) -> p j c", c=256)
                    if b == 0:
                        nc.scalar.mul(vdst, vsrc, VSC / WS)
                    else:
                        nc.vector.tensor_scalar_mul(vdst, vsrc, VSC / WS)

            def spexp_one(b, m8):
                t = T[b]
                sp = ps()
                for nh in range(2):
                    nc.tensor.matmul(
                        nb(sp, nh), t["hnmm"][:, :, m8 * 128:(m8 + 1) * 128],
                        t["gT"][:, :, nh * 512:(nh + 1) * 512],
                        start=True, stop=True, perf_mode=DR)
                nc.scalar.activation(t["expST"][:, m8, :], sp, AF.Exp,
                                     scale=EXPS_S)

            def stage_rsum(b):
                t = T[b]
                rp = ps()
                for nh in range(2):
                    for i in range(4):
                        nc.tensor.matmul(
                            nb(rp, nh), ones2,
                            t["expST"][:, 2 * i:2 * i + 2,
                                       nh * 512:(nh + 1) * 512],
                            start=(i == 0), stop=(i == 3), perf_mode=DR)
                nc.vector.reciprocal_approx_fast(out=t["rinv"], in_=rp)

            def av_half(b, mc):
                # attnV for one output-channel half -> tmp = U/r in fp8
                t = T[b]
                ap2 = ps()
                for i in range(4):
                    for nh in range(2):
                        nc.tensor.matmul(
                            nb(ap2, nh),
                            t["v_nat"][:, 2 * i:2 * i + 2,
                                       mc * 128:(mc + 1) * 128],
                            t["expST"][:, 2 * i:2 * i + 2,
                                       nh * 512:(nh + 1) * 512],
                            start=(i == 0), stop=(i == 3), perf_mode=DR)
                nc.vector.tensor_tensor(t["tmp"][:, mc, :], ap2,
                                        t["rinv"], op=OP.mult)

            def c_qc(b, nh):
                # qc = Wqc hn + Wqc tmp (+ host-folded bias): two moving
                # tensors accumulate into one psum; h2 never materialized.
                t = T[b]
                qp = ps()
                for mc in range(2):
                    nc.tensor.matmul(
                        qp[:, mc * 512:(mc + 1) * 512],
                        wT["wq_c"][:, :, mc, :],
                        t["hnmm"][:, :, nh * 512:(nh + 1) * 512],
                        start=True, stop=False, perf_mode=DR,
                        skip_group_check=True)
                    nc.tensor.matmul(
                        qp[:, mc * 512:(mc + 1) * 512],
                        wT["wq_c"][:, :, mc, :],
                        t["tmp"][:, :, nh * 512:(nh + 1) * 512],
                        start=False, stop=True, perf_mode=DR,
                        skip_group_check=True)
                for mc in range(2):
                    nc.vector.tensor_scalar(
                        out=t["qcT"][:, mc, nh * 512:(nh + 1) * 512],
                        in0=qp[:, mc * 512:(mc + 1) * 512], scalar1=QS / WS,
                        scalar2=cols["bq_c2"][:, mc:mc + 1],
                        op0=OP.mult, op1=OP.add)

            def c_sc(b, nh):
                t = T[b]
                scp = ps()
                nc.tensor.matmul(
                    scp[0:SP, 0:512], t["kcT"][:],
                    t["qcT"][:, :, nh * 512:(nh + 1) * 512],
                    start=True, stop=True, perf_mode=DR)
                nc.scalar.activation(
                    t["expScT"][:, nh * 512:(nh + 1) * 512],
                    scp[0:S, 0:512], AF.Exp, scale=EXPS_C)

            def c_fin(b, nh):
                # rowsum -> rcinv -> escn = E*rcinv -> attnV (already
                # projected values) -> y = psum/HCS + x -> DMA out
                t = T[b]
                esl = t["expScT"][:, nh * 512:(nh + 1) * 512]
                crp = ps()
                nc.tensor.matmul(crp[:, 0:512], onesc, esl,
                                 start=True, stop=True)
                rsl = t["rcinv"][:, nh * 512:(nh + 1) * 512]
                nc.vector.reciprocal_approx_fast(out=rsl, in_=crp[:, 0:512])
                enl = t["escn"][:, nh * 512:(nh + 1) * 512]
                nc.vector.tensor_tensor(
                    enl, esl, t["rcinv"][0:S, nh * 512:(nh + 1) * 512],
                    op=OP.mult)
                hcp = ps()
                for mc in range(2):
                    nc.tensor.matmul(
                        hcp[:, mc * 512:(mc + 1) * 512],
                        t["vcp"][:, mc * 128:(mc + 1) * 128], enl,
                        start=True, stop=True)
                nc.vector.scalar_tensor_tensor(
                    out=t["y_sb"][:, :, nh * 512:(nh + 1) * 512],
                    in0=hcp[:].rearrange("p (m n) -> p m n", n=512),
                    scalar=1.0 / HCS,
                    in1=t["xT"][:, :, nh * 512:(nh + 1) * 512],
                    op0=OP.mult, op1=OP.add)
                eng = nc.sync if nh == 0 else nc.scalar
                eng.dma_start(
                    out=y_d[b].rearrange("(a p) n -> p a n",
                                         p=128)[:, :, nh * 512:(nh + 1) * 512],
                    in_=t["y_sb"][:, :, nh * 512:(nh + 1) * 512])

            gn_stats(0)
            # ======= stage B: cross k / v / projected-v matmuls ========
            for b in range(BPC):
                t = T[b]
                kc_ps = ps()
                for mc in range(2):
                    for i in range(2):
                        nc.tensor.matmul(
                            kc_ps[:, mc * 512:mc * 512 + SP],
                            wT["wk_c"][:, 2 * i:2 * i + 2, mc, :],
                            t["cembT"][:, 2 * i:2 * i + 2, :],
                            start=(i == 0), stop=(i == 1), perf_mode=DR)
                vcT_ps = ps()
                for mc in range(2):
                    for i in range(2):
                        nc.tensor.matmul(
                            vcT_ps[:, mc * 512:mc * 512 + SP],
                            wT["wv_c"][:, 2 * i:2 * i + 2, mc, :],
                            t["cembT"][:, 2 * i:2 * i + 2, :],
                            start=(i == 0), stop=(i == 1), perf_mode=DR)
                # kcT evacs on ACT (idle this early); vc on DVE
                nc.vector.memset(t["kcT"][:, :, S:SP], 0.0)
                nc.vector.memset(t["vc_f8"][:, :, S:SP], 0.0)
                for mc in range(2):
                    nc.scalar.activation(
                        out=t["kcT"][:, mc, 0:S],
                        in_=kc_ps[:, mc * 512:mc * 512 + S],
                        func=AF.Identity,
                        bias=cols["bk_c2"][:, mc:mc + 1], scale=QS / WS)
                    nc.vector.tensor_scalar(
                        out=t["vc_f8"][:, mc, 0:S],
                        in0=vcT_ps[:, mc * 512:mc * 512 + S],
                        scalar1=VS2 / WS,
                        scalar2=cols["bvc_col"][:, mc:mc + 1],
                        op0=OP.mult, op1=OP.add)
                # vcp = vc @ Wp^T + bp on [77, 256] (proj folded into values)
                vcp_ps = ps()
                nc.tensor.matmul(
                    vcp_ps[0:SP, 0:C], ones_row[0:1, 0:SP], bp_sb[0:1, :],
                    start=True, stop=False, skip_group_check=True)
                nc.tensor.matmul(
                    vcp_ps[0:SP, 0:C], t["vc_f8"][:, :, 0:SP], wproj_flat,
                    start=False, stop=True, perf_mode=DR,
                    skip_group_check=True)
                nc.vector.tensor_scalar_mul(
                    t["vcp"], vcp_ps[0:S, 0:C], 1.0 / (VS2 * WS))

            gn_stats(1)
            gn_group(0)
            warm_burst(10)
            gn_group(1)
            gn_bcast(0)
            gn_affine(0)
            warm_burst(3)
            gn_bcast(1)
            gn_affine(1)
            gv(0)
            gv(1)
            warm_burst(3)
            for m8 in range(8):
                spexp_one(0, m8)
            for m8 in range(3):
                spexp_one(1, m8)
            stage_rsum(0)
            spexp_one(1, 3)
            av_half(0, 0)
            spexp_one(1, 4)
            spexp_one(1, 5)
            av_half(0, 1)
            spexp_one(1, 6)
            spexp_one(1, 7)
            c_qc(0, 0)
            c_qc(0, 1)
            stage_rsum(1)
            av_half(1, 0)
            av_half(1, 1)
            c_qc(1, 0)
            c_sc(0, 0)
            c_qc(1, 1)
            c_sc(0, 1)
            c_fin(0, 0)
            c_sc(1, 0)
            c_fin(0, 1)
            c_sc(1, 1)
            c_fin(1, 0)
            warm_burst(2)
            c_fin(1, 1)

    nc.finalize()
    return nc


def host_inputs(inputs):
    import ml_dtypes
    bf16 = ml_dtypes.bfloat16
    fp8 = ml_dtypes.float8_e4m3
    f = lambda a: np.ascontiguousarray(np.asarray(a, dtype=np.float32))
    x = f(inputs["x"]).reshape(B, C, HW).astype(bf16)
    cembT = np.zeros((B, CD // 128, 128, SP), np.float32)
    cembT[:, :, :, :S] = f(inputs["cemb"]).transpose(0, 2, 1).reshape(
        B, CD // 128, 128, S)
    cembT = cembT.astype(fp8)
    gsel = np.zeros((128, 16), np.float32)
    gsel[np.arange(128), np.arange(128) // 8] = 1.0
    wq_s, wk_s = f(inputs["wq_s"]), f(inputs["wk_s"])
    m2 = wq_s.T @ wk_s  # S = (hn M2 + bq Wk) hn^T
    shared = {
        "gn_gamma": f(inputs["gn_gamma"]),
        "gn_beta": f(inputs["gn_beta"]),
        "bg2": GQS * (f(inputs["bq_s"]) @ wk_s),
        "bq_c2": QS * (f(inputs["bq_c"])
                       + f(inputs["bv_s"]) @ f(inputs["wq_c"]).T),
        "bk_c2": QS * f(inputs["bk_c"]),
        "bvc_col": VS2 * f(inputs["bv_c"]),
        "bp_row": np.ascontiguousarray(
            (VS2 * WS * f(inputs["b_proj"])).reshape(1, C)).astype(bf16),
        "gsel": gsel,
        "gbc": np.ascontiguousarray(gsel.T),
    }
    wmats = {
        "m2": GMS * m2,  # already [kin, kout] layout
        "wv_s": WS * f(inputs["wv_s"]).T,
        "wq_c": WS * f(inputs["wq_c"]).T,
        "w_proj": WS * f(inputs["w_proj"]).T,
        "wk_c": WS * f(inputs["wk_c"]).T,
        "wv_c": WS * f(inputs["wv_c"]).T,
    }
    for name, w in wmats.items():
        kin = w.shape[0]
        shared["wT_" + name] = np.ascontiguousarray(
            w.reshape(kin // 128, 128, 2, 128)).astype(fp8)
    return [
        {"xbf": x[i * BPC:(i + 1) * BPC],
         "cembT": cembT[i * BPC:(i + 1) * BPC], **shared}
        for i in range(NCORES)
    ]


def kernel(**inputs):
    global LAST_RESULT
    from concourse.bass_utils import run_bass_kernel_spmd

    if "nc" not in _CACHE:
        _CACHE["nc"] = _build_nc()
    nc = _CACHE["nc"]

    in_maps = host_inputs(inputs)
    res = run_bass_kernel_spmd(nc, in_maps, list(range(NCORES)),
                               trace=bool(os.environ.get("BASS_TRACE")))
    LAST_RESULT = res
    y = np.concatenate([res.results[i]["y"] for i in range(NCORES)], axis=0)
    return y.reshape(B, C, H, W).astype(np.float32)


# revision 4
# speedup vs baseline: 1.1773x; 1.0689x over previous
"""AttnBlock (GroupNorm + self-attn + cross-attn + proj, residual) on 8 trn2 cores.

Sharding: data-parallel over batch B=16 -> 2 images per core; weights replicated.

v5: v4's algebraic restructuring + DMA-descriptor and scheduling fixes.
 - All DRAM layouts are partition-major so every DMA row is contiguous
   (x 2KB rows, weights packed into ONE [128, 16*256] tensor = 8KB rows,
   cemb 320B rows, y 2KB rows, cols packed into one [128,12]). v4 issued
   ~4600 sub-1KB descriptors and the input phase was descriptor-bound.
 - Image 0's GN -> g -> exp chain is scheduled front-to-back while image
   1's GN/gv overlaps image 0's exp stream (ACT runs img0 affine/evacs
   pre-exp; DVE handles img1).
 - proj folded into vc (vcp = vc Wp^T + bp on [77,256]); h2 eliminated
   via two-moving qc accumulation; q/k merged via M2 = Wq^T Wk; cross
   softmax normalized on the [77,512] exp tile; y stored bf16.

Scale ledger (host WS=16 on true weights):
  M2' = GMS*(Wq^T Wk), GMS=128; gT = GQS*g + GQS*(bq Wk), GQS=8
  S^T psum = GQS*logits -> exp(scale=1/(16*GQS))
  v' = VSC*(hn Wv^T), VSC=2 = ones_self -> rinv = 1/(VSC*r), tmp = U/r fp8
  qc psum = WS*(Wqc(hn+tmp)); qcT = QS*qc + QS*(bqc + Wqc bv_s), QS=2
  kcT = QS*kc; sc psum = QS^2*(qc.kc) -> exp(scale=1/64), expScT bf16
  vcT = VS2*vc (VS2=4, bias col VS2*bvc); vcp psum = VS2*WS*(vc Wp^T)+bias
  vcp bf16 natural (evac scale 1/(VS2*WS)); onesc = 1/HCS (HCS=8)
  rcinv = HCS/rc; escn = E*rcinv; hc psum = HCS*out; y = psum/HCS + x
"""

import os

import numpy as np

B, C, H, W, S, CD = 16, 256, 32, 32, 77, 512
HW = H * W
SP = 80  # S padded to a 16B-aligned stride for DoubleRow APs
GROUPS = 32
GS = C // GROUPS
EPS = 1e-5
NCORES = 8
BPC = B // NCORES

WS = 16.0          # host-side weight scale (fp8 subnormal avoidance)
QS = 2.0           # qc/kc storage scale
GMS = 128.0        # host scale on M2 = Wq^T Wk
GQS = 8.0          # gT storage scale
EXPS_S = 1.0 / (16.0 * GQS)    # self exp scale
EXPS_C = 1.0 / (16.0 * QS * QS)  # cross exp scale
VSC = 2.0          # v storage scale == ones_self value
VS2 = 4.0          # vc fp8 storage scale
HCS = 8.0          # ones_cross = 1/HCS; final evac scale 1/HCS

# packed weight layout: name -> (k0, kch) into wall [128, 16, 2, 128]
WPACK = {"m2": (0, 2), "wv_s": (2, 2), "wq_c": (4, 2), "w_proj": (6, 2),
         "wk_c": (8, 4), "wv_c": (12, 4)}
CPACK = ["bk_c2", "bvc_col", "gn_gamma", "gn_beta", "bg2", "bq_c2"]

_CACHE = {}
LAST_RESULT = None  # test harness reads exec_time_ns off this


def _build_nc():
    import concourse.bacc as bacc
    import concourse.bass as bass
    import concourse.tile as tile
    from concourse import mybir

    f32 = mybir.dt.float32
    bf16 = mybir.dt.bfloat16
    fp8 = mybir.dt.float8e4
    AF = mybir.ActivationFunctionType
    OP = mybir.AluOpType
    DR = mybir.MatmulPerfMode.DoubleRow

    nc = bacc.Bacc("TRN2", target_bir_lowering=False, debug=False)

    x_d = nc.dram_tensor("xbf", [BPC, 128, 2, HW], bf16, kind="ExternalInput")
    cembT_d = nc.dram_tensor("cembT", [BPC, 128, 4 * SP], fp8,
                             kind="ExternalInput")
    wall_d = nc.dram_tensor("wall", [128, 16, 2, 128], fp8,
                            kind="ExternalInput")
    cols_d = nc.dram_tensor("cols", [128, 2 * len(CPACK)], f32,
                            kind="ExternalInput")
    bp_d = nc.dram_tensor("bp_row", [1, C], bf16, kind="ExternalInput")
    gsel_d = nc.dram_tensor("gsel", [128, 16], f32, kind="ExternalInput")
    gbc_d = nc.dram_tensor("gbc", [16, 128], f32, kind="ExternalInput")
    y_d = nc.dram_tensor("y", [BPC, 128, 2, 2, 512], bf16,
                         kind="ExternalOutput")

    with tile.TileContext(nc) as tc:
        with (
            tc.tile_pool(name="const", bufs=1) as const,
            tc.tile_pool(name="work", bufs=2) as work,
            tc.tile_pool(name="psp", bufs=3, space="PSUM") as psp,
            tc.tile_pool(name="pgn", bufs=1, space="PSUM") as pgn,
            tc.tile_pool(name="pwu", bufs=1, space="PSUM") as pwu,
        ):
            # ---- constants (no DMA) ----
            ones2 = const.tile([128, 2, 128], fp8)
            nc.vector.memset(ones2, VSC)
            onesc = const.tile([S, 128], bf16)
            nc.vector.memset(onesc, 1.0 / HCS)
            ones_row = const.tile([1, 512], bf16)
            nc.vector.memset(ones_row, 1.0)
            # touch Exp once so its ACT table load overlaps the weight DMAs
            warm = const.tile([128, 1], f32)
            nc.vector.memset(warm, 0.0)
            nc.scalar.activation(warm, warm, AF.Exp)
            # dummy matmuls during the input-DMA window: keeps the PE HAM
            # activity monitor busy so real matmuls start at full clock
            dummy_mov = const.tile([128, 2, 512], fp8)
            nc.vector.memset(dummy_mov, 1.0)
            wup = pwu.tile([128, 512], f32, tag="wup", name="wup")

            def warm_burst(n):
                for i in range(n):
                    nc.tensor.matmul(wup, ones2, dummy_mov,
                                     start=(i == 0), stop=(i == n - 1),
                                     perf_mode=DR)

            warm_burst(10)

            # ---- input DMAs ----
            # sync queue: x first (GN head is the critical path)
            xTs, cembTs = [], []
            for b in range(BPC):
                xT = work.tile([128, 2, HW], bf16, tag="xT")
                for a in range(2):
                    nc.sync.dma_start(out=xT[:, a, :], in_=x_d[b][:, a, :])
                xTs.append(xT)
            # scalar queue: selectors, cemb, packed weights, proj bias
            gsel = const.tile([128, 16], f32)
            nc.scalar.dma_start(out=gsel, in_=gsel_d[:])
            gbc = const.tile([16, 128], f32)
            nc.scalar.dma_start(out=gbc, in_=gbc_d[:])
            for b in range(BPC):
                cembT = work.tile([128, 4, SP], fp8, tag="cembT")
                nc.scalar.dma_start(
                    out=cembT,
                    in_=cembT_d[b].rearrange("p (k s) -> p k s", s=SP))
                cembTs.append(cembT)
            wall = const.tile([128, 16, 2, 128], fp8)
            nc.scalar.dma_start(out=wall, in_=wall_d[:])
            bp_sb = const.tile([1, C], bf16)
            nc.scalar.dma_start(out=bp_sb, in_=bp_d[:])
            # gpsimd queue: packed bias/affine columns
            cols_all = const.tile([128, 2 * len(CPACK)], f32)
            nc.gpsimd.dma_start(out=cols_all, in_=cols_d[:])
            cols = {name: cols_all[:, 2 * i:2 * i + 2]
                    for i, name in enumerate(CPACK)}

            def wsl(name):
                k0, kch = WPACK[name]
                return wall[:, k0:k0 + kch, :, :]

            wvs_flat = wsl("wv_s").rearrange("p k m c -> p k (m c)")
            wproj_flat = wsl("w_proj").rearrange("p k m c -> p k (m c)")

            nb = lambda ap, nh: ap[:, nh * 512:(nh + 1) * 512]

            # per-image tiles
            T = [dict(xT=xTs[b], cembT=cembTs[b]) for b in range(BPC)]
            for b in range(BPC):
                t = T[b]
                for key, shape, dt_ in [
                    ("kcT", [128, 2, SP], fp8),
                    ("vc_f8", [128, 2, SP], fp8),
                    ("vcp", [S, C], bf16),
                    ("stats6", [128, 2, 2, 6], f32),
                    ("qsum", [128, 2, 2], f32),
                    ("m2sum", [128, 2, 2], f32),
                    ("msq_e", [128, 2, 2], f32),
                    ("musq", [128, 2, 2], f32),
                    ("spack", [128, 3, 2, 1], f32),
                    ("tm", [16, 2], f32),
                    ("ex2", [16, 2], f32),
                    ("msq", [16, 2], f32),
                    ("varv", [16, 2], f32),
                    ("ya", [16, 2], f32),
                    ("yb", [16, 2], f32),
                    ("y2", [16, 2], f32),
                    ("mrp", [16, 4], f32),
                    ("Acol", [128, 2], f32),
                    ("Bcol", [128, 2], f32),
                    ("t1", [128, 2], f32),
                    ("hnmm", [128, 2, HW], fp8),
                    ("gT", [128, 2, HW], fp8),
                    ("v_nat", [128, 8, C], fp8),
                    ("expST", [128, 8, HW], fp8),
                    ("rinv", [128, HW], f32),
                    ("tmp", [128, 2, HW], fp8),
                    ("qcT", [128, 2, HW], fp8),
                    ("expScT", [S, HW], bf16),
                    ("escn", [S, HW], bf16),
                    ("rcinv", [128, HW], f32),
                    ("y_sb", [128, 2, HW], bf16),
                ]:
                    t[key] = work.tile(shape, dt_, tag=key, name=key)

            ps = lambda: psp.tile([128, HW], f32, tag="ps", name="ps")

            def gn_stats(b):
                t = T[b]
                AX = mybir.AxisListType
                for a in range(2):
                    for ch in range(2):
                        nc.vector.bn_stats(
                            t["stats6"][:, a, ch, :],
                            t["xT"][:, a, ch * 512:(ch + 1) * 512])
                s6 = t["stats6"]
                m_e, m_o = s6[:, :, :, 1:2], s6[:, :, :, 4:5]
                v_e, v_o = s6[:, :, :, 2:3], s6[:, :, :, 5:6]
                nc.vector.tensor_add(t["qsum"], m_e, m_o)
                nc.vector.tensor_add(t["m2sum"], v_e, v_o)
                nc.vector.tensor_mul(t["msq_e"], m_e, m_e)
                nc.vector.tensor_mul(t["musq"], m_o, m_o)
                nc.vector.tensor_add(t["musq"], t["musq"], t["msq_e"])
                nc.vector.reduce_sum(out=t["spack"][:, 0, :, :],
                                     in_=t["qsum"], axis=AX.X)
                nc.vector.reduce_sum(out=t["spack"][:, 1, :, :],
                                     in_=t["m2sum"], axis=AX.X)
                nc.vector.reduce_sum(out=t["spack"][:, 2, :, :],
                                     in_=t["musq"], axis=AX.X)

            def gn_group(b):
                t = T[b]
                gps = pgn.tile([128, 512], f32, tag="gps", name="gps")
                nc.tensor.matmul(gps[0:16, 0:6], gsel, t["spack"],
                                 start=True, stop=True)
                nc.vector.tensor_scalar_mul(t["mrp"][:, 0:2], gps[0:16, 0:2],
                                            1.0 / 32.0)
                nc.vector.tensor_scalar_mul(t["tm"], gps[0:16, 2:4],
                                            1.0 / 8192.0)
                nc.vector.scalar_tensor_tensor(
                    out=t["ex2"], in0=gps[0:16, 4:6], scalar=1.0 / 32.0,
                    in1=t["tm"], op0=OP.mult, op1=OP.add)
                nc.vector.tensor_mul(t["msq"], t["mrp"][:, 0:2],
                                     t["mrp"][:, 0:2])
                nc.vector.tensor_sub(t["varv"], t["ex2"], t["msq"])
                nc.vector.tensor_scalar_add(t["varv"], t["varv"], EPS)
                nc.vector.reciprocal_approx_fast(out=t["ya"], in_=t["varv"])
                cur = t["ya"]
                for it in range(1):
                    nc.vector.tensor_mul(t["y2"], cur, cur)
                    nc.vector.tensor_mul(t["y2"], t["y2"], t["varv"])
                    nc.vector.tensor_scalar(out=t["y2"], in0=t["y2"],
                                            scalar1=-0.5, scalar2=1.5,
                                            op0=OP.mult, op1=OP.add)
                    nxt = t["yb"] if cur is t["ya"] else t["ya"]
                    nc.vector.tensor_mul(nxt, cur, t["y2"])
                    cur = nxt
                nc.vector.tensor_copy(t["mrp"][:, 2:4], cur)

            def gn_bcast(b):
                t = T[b]
                mps = pgn.tile([128, 512], f32, tag="gps", name="mps")
                nc.tensor.matmul(mps[0:128, 0:4], gbc, t["mrp"],
                                 start=True, stop=True)
                t["mps"] = mps

            def gn_affine(b):
                t = T[b]
                mps = t["mps"]
                nc.vector.tensor_mul(t["Acol"], mps[0:128, 2:4],
                                     cols["gn_gamma"])
                nc.vector.tensor_mul(t["t1"], mps[0:128, 0:2], t["Acol"])
                nc.vector.tensor_sub(t["Bcol"], cols["gn_beta"], t["t1"])
                for a in range(2):
                    if b == 0:
                        nc.scalar.activation(
                            out=t["hnmm"][:, a, :], in_=t["xT"][:, a, :],
                            func=AF.Identity,
                            bias=t["Bcol"][:, a:a + 1],
                            scale=t["Acol"][:, a:a + 1])
                    else:
                        nc.vector.tensor_scalar(
                            out=t["hnmm"][:, a, :], in0=t["xT"][:, a, :],
                            scalar1=t["Acol"][:, a:a + 1],
                            scalar2=t["Bcol"][:, a:a + 1],
                            op0=OP.mult, op1=OP.add)

            def stageB_mm(b):
                t = T[b]
                kc_ps = ps()
                for mc in range(2):
                    for i in range(2):
                        nc.tensor.matmul(
                            kc_ps[:, mc * 512:mc * 512 + SP],
                            wsl("wk_c")[:, 2 * i:2 * i + 2, mc, :],
                            t["cembT"][:, 2 * i:2 * i + 2, :],
                            start=(i == 0), stop=(i == 1), perf_mode=DR)
                vcT_ps = ps()
                for mc in range(2):
                    for i in range(2):
                        nc.tensor.matmul(
                            vcT_ps[:, mc * 512:mc * 512 + SP],
                            wsl("wv_c")[:, 2 * i:2 * i + 2, mc, :],
                            t["cembT"][:, 2 * i:2 * i + 2, :],
                            start=(i == 0), stop=(i == 1), perf_mode=DR)
                t["kc_ps"], t["vcT_ps"] = kc_ps, vcT_ps

            def stageB_evac(b):
                t = T[b]
                nc.vector.memset(t["kcT"][:, :, S:SP], 0.0)
                nc.vector.memset(t["vc_f8"][:, :, S:SP], 0.0)
                for mc in range(2):
                    nc.scalar.activation(
                        out=t["kcT"][:, mc, 0:S],
                        in_=t["kc_ps"][:, mc * 512:mc * 512 + S],
                        func=AF.Identity,
                        bias=cols["bk_c2"][:, mc:mc + 1], scale=QS / WS)
                    nc.vector.tensor_scalar(
                        out=t["vc_f8"][:, mc, 0:S],
                        in0=t["vcT_ps"][:, mc * 512:mc * 512 + S],
                        scalar1=VS2 / WS,
                        scalar2=cols["bvc_col"][:, mc:mc + 1],
                        op0=OP.mult, op1=OP.add)

            def vcp_mm(b):
                # vcp = vc @ Wp^T + bp on [77, 256] (proj folded into values)
                t = T[b]
                vcp_ps = ps()
                nc.tensor.matmul(
                    vcp_ps[0:SP, 0:C], ones_row[0:1, 0:SP], bp_sb[0:1, :],
                    start=True, stop=False, skip_group_check=True)
                nc.tensor.matmul(
                    vcp_ps[0:SP, 0:C], t["vc_f8"][:, :, 0:SP], wproj_flat,
                    start=False, stop=True, perf_mode=DR,
                    skip_group_check=True)
                nc.vector.tensor_scalar_mul(
                    t["vcp"], vcp_ps[0:S, 0:C], 1.0 / (VS2 * WS))

            def gv(b):
                t = T[b]
                for mc in range(2):
                    qp = ps()
                    for nh in range(2):
                        nc.tensor.matmul(
                            nb(qp, nh), wsl("m2")[:, :, mc, :],
                            t["hnmm"][:, :, nh * 512:(nh + 1) * 512],
                            start=True, stop=True, perf_mode=DR)
                    if b == 0:
                        nc.scalar.activation(
                            out=t["gT"][:, mc, :], in_=qp, func=AF.Identity,
                            bias=cols["bg2"][:, mc:mc + 1], scale=GQS / GMS)
                    else:
                        nc.vector.tensor_scalar(
                            out=t["gT"][:, mc, :], in0=qp, scalar1=GQS / GMS,
                            scalar2=cols["bg2"][:, mc:mc + 1],
                            op0=OP.mult, op1=OP.add)
                for half in range(2):
                    vp = ps()
                    for j in range(4):
                        m8 = 4 * half + j
                        nc.tensor.matmul(
                            vp[:, j * 256:(j + 1) * 256],
                            t["hnmm"][:, :, m8 * 128:(m8 + 1) * 128],
                            wvs_flat,
                            start=True, stop=True, perf_mode=DR)
                    vdst = t["v_nat"][:, 4 * half:4 * half + 4, :]
                    vsrc = vp[:].rearrange("p (j c) -> p j c", c=256)
                    if b == 0:
                        nc.scalar.mul(vdst, vsrc, VSC / WS)
                    else:
                        nc.vector.tensor_scalar_mul(vdst, vsrc, VSC / WS)

            def spexp_one(b, m8):
                t = T[b]
                sp = ps()
                for nh in range(2):
                    nc.tensor.matmul(
                        nb(sp, nh), t["hnmm"][:, :, m8 * 128:(m8 + 1) * 128],
                        t["gT"][:, :, nh * 512:(nh + 1) * 512],
                        start=True, stop=True, perf_mode=DR)
                nc.scalar.activation(t["expST"][:, m8, :], sp, AF.Exp,
                                     scale=EXPS_S)

            def stage_rsum(b):
                t = T[b]
                rp = ps()
                for nh in range(2):
                    for i in range(4):
                        nc.tensor.matmul(
                            nb(rp, nh), ones2,
                            t["expST"][:, 2 * i:2 * i + 2,
                                       nh * 512:(nh + 1) * 512],
                            start=(i == 0), stop=(i == 3), perf_mode=DR)
                nc.vector.reciprocal_approx_fast(out=t["rinv"], in_=rp)

            def av_half(b, mc):
                t = T[b]
                ap2 = ps()
                for i in range(4):
                    for nh in range(2):
                        nc.tensor.matmul(
                            nb(ap2, nh),
                            t["v_nat"][:, 2 * i:2 * i + 2,
                                       mc * 128:(mc + 1) * 128],
                            t["expST"][:, 2 * i:2 * i + 2,
                                       nh * 512:(nh + 1) * 512],
                            start=(i == 0), stop=(i == 3), perf_mode=DR)
                nc.vector.tensor_tensor(t["tmp"][:, mc, :], ap2,
                                        t["rinv"], op=OP.mult)

            def c_qc(b, nh):
                t = T[b]
                qp = ps()
                for mc in range(2):
                    nc.tensor.matmul(
                        qp[:, mc * 512:(mc + 1) * 512],
                        wsl("wq_c")[:, :, mc, :],
                        t["hnmm"][:, :, nh * 512:(nh + 1) * 512],
                        start=True, stop=False, perf_mode=DR,
                        skip_group_check=True)
                    nc.tensor.matmul(
                        qp[:, mc * 512:(mc + 1) * 512],
                        wsl("wq_c")[:, :, mc, :],
                        t["tmp"][:, :, nh * 512:(nh + 1) * 512],
                        start=False, stop=True, perf_mode=DR,
                        skip_group_check=True)
                for mc in range(2):
                    nc.vector.tensor_scalar(
                        out=t["qcT"][:, mc, nh * 512:(nh + 1) * 512],
                        in0=qp[:, mc * 512:(mc + 1) * 512], scalar1=QS / WS,
                        scalar2=cols["bq_c2"][:, mc:mc + 1],
                        op0=OP.mult, op1=OP.add)

            def c_sc(b, nh):
                t = T[b]
                scp = ps()
                nc.tensor.matmul(
                    scp[0:SP, 0:512], t["kcT"][:],
                    t["qcT"][:, :, nh * 512:(nh + 1) * 512],
                    start=True, stop=True, perf_mode=DR)
                nc.scalar.activation(
                    t["expScT"][:, nh * 512:(nh + 1) * 512],
                    scp[0:S, 0:512], AF.Exp, scale=EXPS_C)

            def c_fin(b, nh):
                t = T[b]
                esl = t["expScT"][:, nh * 512:(nh + 1) * 512]
                crp = ps()
                nc.tensor.matmul(crp[:, 0:512], onesc, esl,
                                 start=True, stop=True)
                rsl = t["rcinv"][:, nh * 512:(nh + 1) * 512]
                nc.vector.reciprocal_approx_fast(out=rsl, in_=crp[:, 0:512])
                enl = t["escn"][:, nh * 512:(nh + 1) * 512]
                nc.vector.tensor_tensor(
                    enl, esl, t["rcinv"][0:S, nh * 512:(nh + 1) * 512],
                    op=OP.mult)
                hcp = ps()
                for mc in range(2):
                    nc.tensor.matmul(
                        hcp[:, mc * 512:(mc + 1) * 512],
                        t["vcp"][:, mc * 128:(mc + 1) * 128], enl,
                        start=True, stop=True)
                nc.vector.scalar_tensor_tensor(
                    out=t["y_sb"][:, :, nh * 512:(nh + 1) * 512],
                    in0=hcp[:].rearrange("p (m n) -> p m n", n=512),
                    scalar=1.0 / HCS,
                    in1=t["xT"][:, :, nh * 512:(nh + 1) * 512],
                    op0=OP.mult, op1=OP.add)
                eng = nc.sync if nh == 0 else nc.scalar
                eng.dma_start(
                    out=y_d[b][:, nh],
                    in_=t["y_sb"][:, :, nh * 512:(nh + 1) * 512])

            # ================= schedule =================
            gn_stats(0)
            stageB_mm(0)
            stageB_mm(1)
            warm_burst(4)
            stageB_evac(0)
            gn_group(0)
            stageB_evac(1)
            gn_bcast(0)
            gn_affine(0)
            vcp_mm(0)
            vcp_mm(1)
            gn_stats(1)
            gv(0)
            gn_group(1)
            gn_bcast(1)
            gn_affine(1)
            spexp_one(0, 0)
            spexp_one(0, 1)
            gv(1)
            for m8 in range(2, 8):
                spexp_one(0, m8)
            for m8 in range(3):
                spexp_one(1, m8)
            stage_rsum(0)
            spexp_one(1, 3)
            av_half(0, 0)
            spexp_one(1, 4)
            spexp_one(1, 5)
            av_half(0, 1)
            spexp_one(1, 6)
            spexp_one(1, 7)
            c_qc(0, 0)
            c_qc(0, 1)
            stage_rsum(1)
            av_half(1, 0)
            av_half(1, 1)
            c_qc(1, 0)
            c_sc(0, 0)
            c_qc(1, 1)
            c_sc(0, 1)
            c_fin(0, 0)
            c_sc(1, 0)
            c_fin(0, 1)
            c_sc(1, 1)
            c_fin(1, 0)
            warm_burst(2)
            c_fin(1, 1)

    nc.finalize()
    return nc


def host_inputs(inputs):
    import ml_dtypes
    bf16 = ml_dtypes.bfloat16
    fp8 = ml_dtypes.float8_e4m3
    f = lambda a: np.ascontiguousarray(np.asarray(a, dtype=np.float32))
    # x: [B, C, HW] -> [B, 128(p), 2(a), HW] with c = a*128 + p
    x = f(inputs["x"]).reshape(B, 2, 128, HW).transpose(0, 2, 1, 3)
    x = np.ascontiguousarray(x).astype(bf16)
    # cemb: [B, S, CD] -> [B, 128(p), 4(k), SP] with cd = k*128 + p
    cembT = np.zeros((B, 128, 4, SP), np.float32)
    cembT[:, :, :, :S] = f(inputs["cemb"]).transpose(0, 2, 1).reshape(
        B, 4, 128, S).transpose(0, 2, 1, 3)
    cembT = cembT.reshape(B, 128, 4 * SP).astype(fp8)
    gsel = np.zeros((128, 16), np.float32)
    gsel[np.arange(128), np.arange(128) // 8] = 1.0
    wq_s, wk_s = f(inputs["wq_s"]), f(inputs["wk_s"])
    wmats = {
        "m2": GMS * (wq_s.T @ wk_s),  # already [kin, kout] layout
        "wv_s": WS * f(inputs["wv_s"]).T,
        "wq_c": WS * f(inputs["wq_c"]).T,
        "w_proj": WS * f(inputs["w_proj"]).T,
        "wk_c": WS * f(inputs["wk_c"]).T,
        "wv_c": WS * f(inputs["wv_c"]).T,
    }
    # pack: wall [128(p), 16(k), 2(m), 128(c)]; w row index kin = k*128 + p
    wall = np.zeros((128, 16, 2, 128), np.float32)
    for name, (k0, kch) in WPACK.items():
        w = wmats[name]  # [kin, 256]
        wall[:, k0:k0 + kch] = w.reshape(kch, 128, 2, 128).transpose(
            1, 0, 2, 3)
    colv = {
        "gn_gamma": f(inputs["gn_gamma"]),
        "gn_beta": f(inputs["gn_beta"]),
        "bg2": GQS * (f(inputs["bq_s"]) @ wk_s),
        "bq_c2": QS * (f(inputs["bq_c"])
                       + f(inputs["bv_s"]) @ f(inputs["wq_c"]).T),
        "bk_c2": QS * f(inputs["bk_c"]),
        "bvc_col": VS2 * f(inputs["bv_c"]),
    }
    # cols [128(p), 2*i + a] with c = a*128 + p
    cols = np.zeros((128, 2 * len(CPACK)), np.float32)
    for i, name in enumerate(CPACK):
        cols[:, 2 * i:2 * i + 2] = colv[name].reshape(2, 128).T
    shared = {
        "wall": np.ascontiguousarray(wall).astype(fp8),
        "cols": cols,
        "bp_row": np.ascontiguousarray(
            (VS2 * WS * f(inputs["b_proj"])).reshape(1, C)).astype(bf16),
        "gsel": gsel,
        "gbc": np.ascontiguousarray(gsel.T),
    }
    return [
        {"xbf": x[i * BPC:(i + 1) * BPC],
         "cembT": cembT[i * BPC:(i + 1) * BPC], **shared}
        for i in range(NCORES)
    ]


def kernel(**inputs):
    global LAST_RESULT
    from concourse.bass_utils import run_bass_kernel_spmd

    if "nc" not in _CACHE:
        _CACHE["nc"] = _build_nc()
    nc = _CACHE["nc"]

    in_maps = host_inputs(inputs)
    res = run_bass_kernel_spmd(nc, in_maps, list(range(NCORES)),
                               trace=bool(os.environ.get("BASS_TRACE")))
    LAST_RESULT = res
    # y [BPC, 128(p), 2(nh), 2(a), 512] -> [BPC, C = a*128+p, HW = nh*512+n]
    y = np.concatenate([res.results[i]["y"] for i in range(NCORES)], axis=0)
    y = y.transpose(0, 3, 1, 2, 4).reshape(B, C, HW)
    return y.reshape(B, C, H, W).astype(np.float32)


# revision 7
# speedup vs baseline: 1.2817x; 1.0887x over previous
"""AttnBlock (GroupNorm + self-attn + cross-attn + proj, residual) on 8 trn2 cores.

Sharding: data-parallel over batch B=16 -> 2 images per core; weights replicated.

v6: v5 + critical-path surgery.
 - head: x DMA gets the rings first (weights trail on the same queue);
   affine(0) and gT(0) evacuations split ACT/DVE so the serial
   GN -> affine -> g -> exp chain crosses engines in parallel halves.
 - kc/vc biases ride tiny PE bias-matmuls (brow stationary rows) so the
   stage-B evacuations collapse to one instruction each; kc+vcT share
   one PSUM bank per image.
 - gv(1) is emitted after image 0's exp stream starts; image 1's whole
   GN/gv pipeline hides under image 0's 8 exps.
 - tail: c_fin split into crp/rcinv/escn (escn on the idle GPSIMD) and
   attnV/y/dma stages, interleaved across the four (b, nh) pairs so the
   four cross-attn chains pipeline instead of running serially.
 - PSUM: 3x2-bank pool for the wide matmuls + 2x1-bank pool (warmup,
   groupnorm selector/broadcast, kc/vc/vcp, sc, crossrowsum).

Scale ledger (host WS=16 on true weights):
  M2' = GMS*(Wq^T Wk), GMS=128; gT = GQS*g + GQS*(bq Wk), GQS=8
  S^T psum = GQS*logits -> exp(scale=1/(16*GQS))
  v' = VSC*(hn Wv^T), VSC=2 = ones_self -> rinv = 1/(VSC*r), tmp = U/r fp8
  qc psum = WS*(Wqc(hn+tmp)); qcT = QS*qc + QS*(bqc + Wqc bv_s), QS=2
  kc psum = WS*(kc+bkc) (bias matmul) -> kcT = QS*(kc+bkc)
  vc psum = WS*(vc+bvc) -> vc_f8 = VS2*vc'; vcp psum = VS2*WS*(vc Wp^T)+bp
  vcp bf16 natural; onesc = 1/HCS (HCS=8)
  rcinv = HCS/rc; escn = E*rcinv (gpsimd); hc psum = HCS*out; y = psum/HCS + x
"""

import os

import numpy as np

B, C, H, W, S, CD = 16, 256, 32, 32, 77, 512
HW = H * W
SP = 80  # S padded to a 16B-aligned stride for DoubleRow APs
GROUPS = 32
GS = C // GROUPS
EPS = 1e-5
NCORES = 8
BPC = B // NCORES

WS = 16.0          # host-side weight scale (fp8 subnormal avoidance)
QS = 2.0           # qc/kc storage scale
GMS = 128.0        # host scale on M2 = Wq^T Wk
GQS = 8.0          # gT storage scale
EXPS_S = 1.0 / (16.0 * GQS)    # self exp scale
EXPS_C = 1.0 / (16.0 * QS * QS)  # cross exp scale
VSC = 2.0          # v storage scale == ones_self value
VS2 = 4.0          # vc fp8 storage scale
HCS = 8.0          # ones_cross = 1/HCS; final evac scale 1/HCS

# packed weight layout: name -> (k0, kch) into wall [128, 16, 2, 128]
WPACK = {"m2": (0, 2), "wv_s": (2, 2), "wq_c": (4, 2), "w_proj": (6, 2),
         "wk_c": (8, 4), "wv_c": (12, 4)}
CPACK = ["gn_gamma", "gn_beta", "bg2", "bq_c2"]

_CACHE = {}
LAST_RESULT = None  # test harness reads exec_time_ns off this


def _build_nc():
    import concourse.bacc as bacc
    import concourse.bass as bass
    import concourse.tile as tile
    from concourse import mybir

    f32 = mybir.dt.float32
    bf16 = mybir.dt.bfloat16
    fp8 = mybir.dt.float8e4
    AF = mybir.ActivationFunctionType
    OP = mybir.AluOpType
    DR = mybir.MatmulPerfMode.DoubleRow

    nc = bacc.Bacc("TRN2", target_bir_lowering=False, debug=False)

    x_d = nc.dram_tensor("xbf", [BPC, 128, 2, HW], bf16, kind="ExternalInput")
    cembT_d = nc.dram_tensor("cembT", [BPC, 128, 4 * SP], fp8,
                             kind="ExternalInput")
    wall_d = nc.dram_tensor("wall", [128, 16, 2, 128], fp8,
                            kind="ExternalInput")
    cols_d = nc.dram_tensor("cols", [128, 2 * len(CPACK)], f32,
                            kind="ExternalInput")
    brow_d = nc.dram_tensor("brow", [3, C], bf16, kind="ExternalInput")
    gsel_d = nc.dram_tensor("gsel", [128, 16], f32, kind="ExternalInput")
    gbc_d = nc.dram_tensor("gbc", [16, 128], f32, kind="ExternalInput")
    y_d = nc.dram_tensor("y", [BPC, 128, 2, 2, 512], bf16,
                         kind="ExternalOutput")

    with tile.TileContext(nc) as tc:
        with (
            tc.tile_pool(name="const", bufs=1) as const,
            tc.tile_pool(name="work", bufs=2) as work,
            tc.tile_pool(name="psp", bufs=3, space="PSUM") as psp,
            tc.tile_pool(name="pss", bufs=2, space="PSUM") as pss,
        ):
            # ---- constants (no DMA) ----
            ones2 = const.tile([128, 2, 128], fp8)
            nc.vector.memset(ones2, VSC)
            onesc = const.tile([S, 128], bf16)
            nc.vector.memset(onesc, 1.0 / HCS)
            ones_row = const.tile([1, 512], bf16)
            nc.vector.memset(ones_row, 1.0)
            # touch Exp once so its ACT table load overlaps the weight DMAs
            warm = const.tile([128, 1], f32)
            nc.vector.memset(warm, 0.0)
            nc.scalar.activation(warm, warm, AF.Exp)
            dummy_mov = const.tile([128, 2, 512], fp8)
            nc.vector.memset(dummy_mov, 1.0)

            ps = lambda: psp.tile([128, HW], f32, tag="ps", name="ps")
            ps5 = lambda: pss.tile([128, 512], f32, tag="sm", name="sm")

            def warm_burst(n):
                wup = ps5()
                for i in range(n):
                    nc.tensor.matmul(wup, ones2, dummy_mov,
                                     start=(i == 0), stop=(i == n - 1),
                                     perf_mode=DR)

            warm_burst(10)

            # ---- input DMAs ----
            # sync queue: x first (GN head is the critical path), then the
            # packed weights; scalar queue: selectors + cemb + bias rows
            xTs, cembTs = [], []
            for b in range(BPC):
                xT = work.tile([128, 2, HW], bf16, tag="xT")
                for a in range(2):
                    nc.sync.dma_start(out=xT[:, a, :], in_=x_d[b][:, a, :])
                xTs.append(xT)
            wall = const.tile([128, 16, 2, 128], fp8)
            nc.sync.dma_start(out=wall, in_=wall_d[:])
            gsel = const.tile([128, 16], f32)
            nc.scalar.dma_start(out=gsel, in_=gsel_d[:])
            gbc = const.tile([16, 128], f32)
            nc.scalar.dma_start(out=gbc, in_=gbc_d[:])
            for b in range(BPC):
                cembT = work.tile([128, 4, SP], fp8, tag="cembT")
                nc.scalar.dma_start(
                    out=cembT,
                    in_=cembT_d[b].rearrange("p (k s) -> p k s", s=SP))
                cembTs.append(cembT)
            brows = []
            for i in range(3):
                r = const.tile([1, C], bf16, tag=f"brow{i}")
                nc.scalar.dma_start(out=r, in_=brow_d[i:i + 1, :])
                brows.append(r)
            # gpsimd queue: packed bias/affine columns
            cols_all = const.tile([128, 2 * len(CPACK)], f32)
            nc.gpsimd.dma_start(out=cols_all, in_=cols_d[:])
            cols = {name: cols_all[:, 2 * i:2 * i + 2]
                    for i, name in enumerate(CPACK)}

            def wsl(name):
                k0, kch = WPACK[name]
                return wall[:, k0:k0 + kch, :, :]

            wvs_flat = wsl("wv_s").rearrange("p k m c -> p k (m c)")
            wproj_flat = wsl("w_proj").rearrange("p k m c -> p k (m c)")

            nb = lambda ap, nh: ap[:, nh * 512:(nh + 1) * 512]

            # per-image tiles
            T = [dict(xT=xTs[b], cembT=cembTs[b]) for b in range(BPC)]
            for b in range(BPC):
                t = T[b]
                for key, shape, dt_ in [
                    ("kcT", [128, 2, SP], fp8),
                    ("vc_f8", [128, 2, SP], fp8),
                    ("vcp", [S, C], bf16),
                    ("stats6", [128, 2, 2, 6], f32),
                    ("qsum", [128, 2, 2], f32),
                    ("m2sum", [128, 2, 2], f32),
                    ("msq_e", [128, 2, 2], f32),
                    ("musq", [128, 2, 2], f32),
                    ("spack", [128, 3, 2, 1], f32),
                    ("tm", [16, 2], f32),
                    ("ex2", [16, 2], f32),
                    ("msq", [16, 2], f32),
                    ("varv", [16, 2], f32),
                    ("ya", [16, 2], f32),
                    ("yb", [16, 2], f32),
                    ("y2", [16, 2], f32),
                    ("mrp", [16, 4], f32),
                    ("Acol", [128, 2], f32),
                    ("Bcol", [128, 2], f32),
                    ("t1", [128, 2], f32),
                    ("hnmm", [128, 2, HW], fp8),
                    ("gT", [128, 2, HW], fp8),
                    ("v_nat", [128, 8, C], fp8),
                    ("expST", [128, 8, HW], fp8),
                    ("rinv", [128, HW], f32),
                    ("tmp", [128, 2, HW], fp8),
                    ("qcT", [128, 2, HW], fp8),
                    ("expScT", [S, HW], bf16),
                    ("escn", [S, HW], bf16),
                    ("rcinv", [128, HW], f32),
                    ("y_sb", [128, 2, HW], bf16),
                ]:
                    t[key] = work.tile(shape, dt_, tag=key, name=key)

            def gn_stats(b):
                t = T[b]
                AX = mybir.AxisListType
                for a in range(2):
                    for ch in range(2):
                        nc.vector.bn_stats(
                            t["stats6"][:, a, ch, :],
                            t["xT"][:, a, ch * 512:(ch + 1) * 512])
                s6 = t["stats6"]
                m_e, m_o = s6[:, :, :, 1:2], s6[:, :, :, 4:5]
                v_e, v_o = s6[:, :, :, 2:3], s6[:, :, :, 5:6]
                nc.vector.tensor_add(t["qsum"], m_e, m_o)
                nc.vector.tensor_add(t["m2sum"], v_e, v_o)
                nc.vector.tensor_mul(t["msq_e"], m_e, m_e)
                nc.vector.tensor_mul(t["musq"], m_o, m_o)
                nc.vector.tensor_add(t["musq"], t["musq"], t["msq_e"])
                nc.vector.reduce_sum(out=t["spack"][:, 0, :, :],
                                     in_=t["qsum"], axis=AX.X)
                nc.vector.reduce_sum(out=t["spack"][:, 1, :, :],
                                     in_=t["m2sum"], axis=AX.X)
                nc.vector.reduce_sum(out=t["spack"][:, 2, :, :],
                                     in_=t["musq"], axis=AX.X)

            def gn_group(b):
                t = T[b]
                gps = ps5()
                nc.tensor.matmul(gps[0:16, 0:6], gsel, t["spack"],
                                 start=True, stop=True)
                nc.vector.tensor_scalar_mul(t["mrp"][:, 0:2], gps[0:16, 0:2],
                                            1.0 / 32.0)
                nc.vector.tensor_scalar_mul(t["tm"], gps[0:16, 2:4],
                                            1.0 / 8192.0)
                nc.vector.scalar_tensor_tensor(
                    out=t["ex2"], in0=gps[0:16, 4:6], scalar=1.0 / 32.0,
                    in1=t["tm"], op0=OP.mult, op1=OP.add)
                nc.vector.tensor_mul(t["msq"], t["mrp"][:, 0:2],
                                     t["mrp"][:, 0:2])
                nc.vector.tensor_sub(t["varv"], t["ex2"], t["msq"])
                nc.vector.tensor_scalar_add(t["varv"], t["varv"], EPS)
                nc.vector.reciprocal_approx_fast(out=t["ya"], in_=t["varv"])
                cur = t["ya"]
                for it in range(1):
                    nc.vector.tensor_mul(t["y2"], cur, cur)
                    nc.vector.tensor_mul(t["y2"], t["y2"], t["varv"])
                    nc.vector.tensor_scalar(out=t["y2"], in0=t["y2"],
                                            scalar1=-0.5, scalar2=1.5,
                                            op0=OP.mult, op1=OP.add)
                    nxt = t["yb"] if cur is t["ya"] else t["ya"]
                    nc.vector.tensor_mul(nxt, cur, t["y2"])
                    cur = nxt
                nc.vector.tensor_copy(t["mrp"][:, 2:4], cur)

            def gn_bcast(b):
                t = T[b]
                mps = ps5()
                nc.tensor.matmul(mps[0:128, 0:4], gbc, t["mrp"],
                                 start=True, stop=True)
                t["mps"] = mps

            def gn_affine(b):
                # image 0: a=0 on ACT, a=1 on DVE (parallel halves)
                t = T[b]
                mps = t["mps"]
                nc.vector.tensor_mul(t["Acol"], mps[0:128, 2:4],
                                     cols["gn_gamma"])
                nc.vector.tensor_mul(t["t1"], mps[0:128, 0:2], t["Acol"])
                nc.vector.tensor_sub(t["Bcol"], cols["gn_beta"], t["t1"])
                for a in range(2):
                    if b == 0 and a == 0:
                        nc.scalar.activation(
                            out=t["hnmm"][:, a, :], in_=t["xT"][:, a, :],
                            func=AF.Identity,
                            bias=t["Bcol"][:, a:a + 1],
                            scale=t["Acol"][:, a:a + 1])
                    else:
                        nc.vector.tensor_scalar(
                            out=t["hnmm"][:, a, :], in0=t["xT"][:, a, :],
                            scalar1=t["Acol"][:, a:a + 1],
                            scalar2=t["Bcol"][:, a:a + 1],
                            op0=OP.mult, op1=OP.add)

            def stageB_mm(b):
                # kc and vcT share one 1-bank psum; biases via tiny matmuls
                t = T[b]
                bg = ps5()
                t["bigB"] = bg
                for w, wname, brow_i in ((0, "wk_c", 1), (1, "wv_c", 2)):
                    for mc in range(2):
                        dst = bg[:, (2 * w + mc) * 128:(2 * w + mc) * 128 + SP]
                        nc.tensor.matmul(
                            dst, brows[brow_i][0:1,
                                               mc * 128:(mc + 1) * 128],
                            ones_row[0:1, 0:SP],
                            start=True, stop=False, skip_group_check=True)
                        for i in range(2):
                            nc.tensor.matmul(
                                dst, wsl(wname)[:, 2 * i:2 * i + 2, mc, :],
                                t["cembT"][:, 2 * i:2 * i + 2, :],
                                start=False, stop=(i == 1), perf_mode=DR,
                                skip_group_check=True)

            def stageB_evac(b):
                t = T[b]
                bg = t["bigB"]
                nc.vector.memset(t["kcT"][:, :, S:SP], 0.0)
                nc.vector.memset(t["vc_f8"][:, :, S:SP], 0.0)
                src = bg[:].rearrange("p (g s) -> p g s", s=128)
                nc.scalar.mul(t["kcT"][:, :, 0:S], src[:, 0:2, 0:S], QS / WS)
                nc.vector.tensor_scalar_mul(
                    t["vc_f8"][:, :, 0:S], src[:, 2:4, 0:S], VS2 / WS)

            def vcp_mm(b):
                # vcp = vc @ Wp^T + bp on [77, 256] (proj folded into values)
                t = T[b]
                vcp_ps = ps5()
                nc.tensor.matmul(
                    vcp_ps[0:SP, 0:C], ones_row[0:1, 0:SP], brows[0][0:1, :],
                    start=True, stop=False, skip_group_check=True)
                nc.tensor.matmul(
                    vcp_ps[0:SP, 0:C], t["vc_f8"][:, :, 0:SP], wproj_flat,
                    start=False, stop=True, perf_mode=DR,
                    skip_group_check=True)
                nc.vector.tensor_scalar_mul(
                    t["vcp"], vcp_ps[0:S, 0:C], 1.0 / (VS2 * WS))

            def gv_g(b):
                # g = hn M2 + bg; image 0 evacs split ACT/DVE
                t = T[b]
                for mc in range(2):
                    qp = ps()
                    for nh in range(2):
                        nc.tensor.matmul(
                            nb(qp, nh), wsl("m2")[:, :, mc, :],
                            t["hnmm"][:, :, nh * 512:(nh + 1) * 512],
                            start=True, stop=True, perf_mode=DR)
                    if b == 0 and mc == 0:
                        nc.scalar.activation(
                            out=t["gT"][:, mc, :], in_=qp, func=AF.Identity,
                            bias=cols["bg2"][:, mc:mc + 1], scale=GQS / GMS)
                    else:
                        nc.vector.tensor_scalar(
                            out=t["gT"][:, mc, :], in0=qp, scalar1=GQS / GMS,
                            scalar2=cols["bg2"][:, mc:mc + 1],
                            op0=OP.mult, op1=OP.add)

            def gv_v(b):
                t = T[b]
                for half in range(2):
                    vp = ps()
                    for j in range(4):
                        m8 = 4 * half + j
                        nc.tensor.matmul(
                            vp[:, j * 256:(j + 1) * 256],
                            t["hnmm"][:, :, m8 * 128:(m8 + 1) * 128],
                            wvs_flat,
                            start=True, stop=True, perf_mode=DR)
                    vdst = t["v_nat"][:, 4 * half:4 * half + 4, :]
                    vsrc = vp[:].rearrange("p (j c) -> p j c", c=256)
                    nc.vector.tensor_scalar_mul(vdst, vsrc, VSC / WS)

            def spexp_one(b, m8):
                t = T[b]
                sp = ps()
                for nh in range(2):
                    nc.tensor.matmul(
                        nb(sp, nh), t["hnmm"][:, :, m8 * 128:(m8 + 1) * 128],
                        t["gT"][:, :, nh * 512:(nh + 1) * 512],
                        start=True, stop=True, perf_mode=DR)
                nc.scalar.activation(t["expST"][:, m8, :], sp, AF.Exp,
                                     scale=EXPS_S)

            def stage_rsum(b):
                t = T[b]
                rp = ps()
                for nh in range(2):
                    for i in range(4):
                        nc.tensor.matmul(
                            nb(rp, nh), ones2,
                            t["expST"][:, 2 * i:2 * i + 2,
                                       nh * 512:(nh + 1) * 512],
                            start=(i == 0), stop=(i == 3), perf_mode=DR)
                nc.vector.reciprocal_approx_fast(out=t["rinv"], in_=rp)

            def av_half(b, mc):
                t = T[b]
                ap2 = ps()
                for i in range(4):
                    for nh in range(2):
                        nc.tensor.matmul(
                            nb(ap2, nh),
                            t["v_nat"][:, 2 * i:2 * i + 2,
                                       mc * 128:(mc + 1) * 128],
                            t["expST"][:, 2 * i:2 * i + 2,
                                       nh * 512:(nh + 1) * 512],
                            start=(i == 0), stop=(i == 3), perf_mode=DR)
                nc.vector.tensor_tensor(t["tmp"][:, mc, :], ap2,
                                        t["rinv"], op=OP.mult)

            def c_qc(b, nh):
                t = T[b]
                qp = ps()
                for mc in range(2):
                    nc.tensor.matmul(
                        qp[:, mc * 512:(mc + 1) * 512],
                        wsl("wq_c")[:, :, mc, :],
                        t["hnmm"][:, :, nh * 512:(nh + 1) * 512],
                        start=True, stop=False, perf_mode=DR,
                        skip_group_check=True)
                    nc.tensor.matmul(
                        qp[:, mc * 512:(mc + 1) * 512],
                        wsl("wq_c")[:, :, mc, :],
                        t["tmp"][:, :, nh * 512:(nh + 1) * 512],
                        start=False, stop=True, perf_mode=DR,
                        skip_group_check=True)
                for mc in range(2):
                    nc.vector.tensor_scalar(
                        out=t["qcT"][:, mc, nh * 512:(nh + 1) * 512],
                        in0=qp[:, mc * 512:(mc + 1) * 512], scalar1=QS / WS,
                        scalar2=cols["bq_c2"][:, mc:mc + 1],
                        op0=OP.mult, op1=OP.add)

            def c_sc(b, nh):
                t = T[b]
                scp = ps5()
                nc.tensor.matmul(
                    scp[0:SP, 0:512], t["kcT"][:],
                    t["qcT"][:, :, nh * 512:(nh + 1) * 512],
                    start=True, stop=True, perf_mode=DR)
                nc.scalar.activation(
                    t["expScT"][:, nh * 512:(nh + 1) * 512],
                    scp[0:S, 0:512], AF.Exp, scale=EXPS_C)

            def c_fin_a(b, nh):
                # cross rowsum -> rcinv (DVE) -> escn = E*rcinv (GPSIMD)
                t = T[b]
                esl = t["expScT"][:, nh * 512:(nh + 1) * 512]
                crp = ps5()
                nc.tensor.matmul(crp[:, 0:512], onesc, esl,
                                 start=True, stop=True)
                rsl = t["rcinv"][:, nh * 512:(nh + 1) * 512]
                nc.vector.reciprocal_approx_fast(out=rsl, in_=crp[:, 0:512])
                nc.gpsimd.tensor_tensor(
                    t["escn"][:, nh * 512:(nh + 1) * 512], esl,
                    t["rcinv"][0:S, nh * 512:(nh + 1) * 512], op=OP.mult)

            def c_fin_b(b, nh):
                # attnV over projected values -> y = psum/HCS + x -> DMA out
                t = T[b]
                enl = t["escn"][:, nh * 512:(nh + 1) * 512]
                hcp = ps()
                for mc in range(2):
                    nc.tensor.matmul(
                        hcp[:, mc * 512:(mc + 1) * 512],
                        t["vcp"][:, mc * 128:(mc + 1) * 128], enl,
                        start=True, stop=True)
                nc.vector.scalar_tensor_tensor(
                    out=t["y_sb"][:, :, nh * 512:(nh + 1) * 512],
                    in0=hcp[:].rearrange("p (m n) -> p m n", n=512),
                    scalar=1.0 / HCS,
                    in1=t["xT"][:, :, nh * 512:(nh + 1) * 512],
                    op0=OP.mult, op1=OP.add)
                eng = nc.sync if nh == 0 else nc.scalar
                eng.dma_start(
                    out=y_d[b][:, nh],
                    in_=t["y_sb"][:, :, nh * 512:(nh + 1) * 512])

            # ================= schedule =================
            gn_stats(0)
            gn_group(0)
            gn_bcast(0)
            gn_affine(0)
            stageB_mm(0)
            stageB_mm(1)
            gv_g(0)
            stageB_evac(0)
            stageB_evac(1)
            gn_stats(1)
            vcp_mm(0)
            vcp_mm(1)
            gv_v(0)
            for m8 in range(4):
                spexp_one(0, m8)
            gn_group(1)
            for m8 in range(4, 8):
                spexp_one(0, m8)
            gn_bcast(1)
            gn_affine(1)
            gv_g(1)
            gv_v(1)
            for m8 in range(3):
                spexp_one(1, m8)
            stage_rsum(0)
            spexp_one(1, 3)
            av_half(0, 0)
            spexp_one(1, 4)
            spexp_one(1, 5)
            av_half(0, 1)
            spexp_one(1, 6)
            spexp_one(1, 7)
            c_qc(0, 0)
            c_qc(0, 1)
            stage_rsum(1)
            av_half(1, 0)
            av_half(1, 1)
            c_qc(1, 0)
            c_qc(1, 1)
            c_sc(0, 0)
            c_sc(0, 1)
            c_fin_a(0, 0)
            c_sc(1, 0)
            c_fin_a(0, 1)
            c_sc(1, 1)
            c_fin_b(0, 0)
            c_fin_a(1, 0)
            c_fin_b(0, 1)
            c_fin_a(1, 1)
            c_fin_b(1, 0)
            c_fin_b(1, 1)

    nc.finalize()
    return nc


def host_inputs(inputs):
    import ml_dtypes
    bf16 = ml_dtypes.bfloat16
    fp8 = ml_dtypes.float8_e4m3
    f = lambda a: np.ascontiguousarray(np.asarray(a, dtype=np.float32))
    # x: [B, C, HW] -> [B, 128(p), 2(a), HW] with c = a*128 + p
    x = f(inputs["x"]).reshape(B, 2, 128, HW).transpose(0, 2, 1, 3)
    x = np.ascontiguousarray(x).astype(bf16)
    # cemb: [B, S, CD] -> [B, 128(p), 4(k), SP] with cd = k*128 + p
    cembT = np.zeros((B, 128, 4, SP), np.float32)
    cembT[:, :, :, :S] = f(inputs["cemb"]).transpose(0, 2, 1).reshape(
        B, 4, 128, S).transpose(0, 2, 1, 3)
    cembT = cembT.reshape(B, 128, 4 * SP).astype(fp8)
    gsel = np.zeros((128, 16), np.float32)
    gsel[np.arange(128), np.arange(128) // 8] = 1.0
    wq_s, wk_s = f(inputs["wq_s"]), f(inputs["wk_s"])
    wmats = {
        "m2": GMS * (wq_s.T @ wk_s),  # already [kin, kout] layout
        "wv_s": WS * f(inputs["wv_s"]).T,
        "wq_c": WS * f(inputs["wq_c"]).T,
        "w_proj": WS * f(inputs["w_proj"]).T,
        "wk_c": WS * f(inputs["wk_c"]).T,
        "wv_c": WS * f(inputs["wv_c"]).T,
    }
    # pack: wall [128(p), 16(k), 2(m), 128(c)]; w row index kin = k*128 + p
    wall = np.zeros((128, 16, 2, 128), np.float32)
    for name, (k0, kch) in WPACK.items():
        w = wmats[name]  # [kin, 256]
        wall[:, k0:k0 + kch] = w.reshape(kch, 128, 2, 128).transpose(
            1, 0, 2, 3)
    colv = {
        "gn_gamma": f(inputs["gn_gamma"]),
        "gn_beta": f(inputs["gn_beta"]),
        "bg2": GQS * (f(inputs["bq_s"]) @ wk_s),
        "bq_c2": QS * (f(inputs["bq_c"])
                       + f(inputs["bv_s"]) @ f(inputs["wq_c"]).T),
    }
    # cols [128(p), 2*i + a] with c = a*128 + p
    cols = np.zeros((128, 2 * len(CPACK)), np.float32)
    for i, name in enumerate(CPACK):
        cols[:, 2 * i:2 * i + 2] = colv[name].reshape(2, 128).T
    brow = np.stack([
        VS2 * WS * f(inputs["b_proj"]),
        WS * f(inputs["bk_c"]),
        WS * f(inputs["bv_c"]),
    ]).astype(bf16)
    shared = {
        "wall": np.ascontiguousarray(wall).astype(fp8),
        "cols": cols,
        "brow": np.ascontiguousarray(brow),
        "gsel": gsel,
        "gbc": np.ascontiguousarray(gsel.T),
    }
    return [
        {"xbf": x[i * BPC:(i + 1) * BPC],
         "cembT": cembT[i * BPC:(i + 1) * BPC], **shared}
        for i in range(NCORES)
    ]


def kernel(**inputs):
    global LAST_RESULT
    from concourse.bass_utils import run_bass_kernel_spmd

    if "nc" not in _CACHE:
        _CACHE["nc"] = _build_nc()
    nc = _CACHE["nc"]

    in_maps = host_inputs(inputs)
    res = run_bass_kernel_spmd(nc, in_maps, list(range(NCORES)),
                               trace=bool(os.environ.get("BASS_TRACE")))
    LAST_RESULT = res
    # y [BPC, 128(p), 2(nh), 2(a), 512] -> [BPC, C = a*128+p, HW = nh*512+n]
    y = np.concatenate([res.results[i]["y"] for i in range(NCORES)], axis=0)
    y = y.transpose(0, 3, 1, 2, 4).reshape(B, C, HW)
    return y.reshape(B, C, H, W).astype(np.float32)
